# revision 3
# baseline (speedup 1.0000x reference)
"""Multi-head attention Trainium2 kernel (nn_MultiHeadAttention_86423331930281).

Data-parallel over batch (B=8 -> one batch element per NeuronCore).
Inputs are marshalled on the host into device-friendly layouts (bf16
transposed activations [d, s]; pair-major weight stacks; Wo^T), which is
one-time layout/sharding prep -- all matmuls, softmax and normalization
run on-device.

Per-core plan (S=1024, D=1024, H=16, E=64), all bf16 matmuls:
  - Q/K-proj per head-pair m: QT_m/KT_m [128 = 2 heads x 64e, s] with
    weight slices stationary, qT/kT moving (N=512)
  - scoresT_hj [t_j=128, s=1024] = KT_h j-slice^T @ QT_h (K=64), exp on
    ScalarE (scale 1/32 folded in) -> P_hj [t, s] bf16
  - attended in [s, he] layout: att_ps_h[:, i, :] += P_hj[:, i]^T @ V1_j
    (V moving, N=64/head) accumulated over t-tiles j; denominators via
    near-free N=1 matmuls against a ones column into a shared psum bank
    (start_tensor_calc wipes a whole 2KB bank, so each bank gets exactly
    one start per accumulation round; the den bank is started by a
    dep-visible zeroing matmul per pair)
  - normalize with per-partition reciprocal scalars (DVE tensor_scalar)
  - attT via PE transposes, FC: out = attT^T @ woT + bo
  - the steady state pipelines head pairs: pair m's scores/exp (ACT) are
    overlapped with pair m+1's projections and V-projections on PE
"""

import numpy as np
from contextlib import ExitStack

import concourse.bass as bass
import concourse.mybir as mybir
import concourse.tile as tile
from concourse.bass_utils import run_bass_kernel_spmd
from concourse.masks import make_identity

P = 128
S = 1024          # sequence length
DK = 1024         # qkv input dim
H = 16            # heads
E = 64            # per-head dim
HE = H * E        # 1024
OUT = 1024        # output dim
NT = S // P       # 8 t-tiles
NK = DK // P      # 8 contraction (d) blocks
NM = H // 2       # 8 head pairs
NS = S // P       # 8 s-tiles
F32 = mybir.dt.float32
BF16 = mybir.dt.bfloat16
AF = mybir.ActivationFunctionType
ALU = mybir.AluOpType
SCALE = 1.0 / 32.0  # 1/sqrt(DK)


def _legalize_matmul_waits(nc):
    """This walrus build allows only ONE sync-wait command per Matmult.
    Move all but the last wait of any multi-wait matmul onto freshly
    inserted PE nops immediately before it — same engine queue, so the
    blocking semantics are identical."""
    SKIP = ("NoOp", "Br", "Halt", "Sem", "Event")
    k = 0
    for f in nc.m.functions:
        for b in f.blocks:
            out = []
            for inst in b.instructions:
                si = getattr(inst, "sync_info", None)
                tname = type(inst).__name__
                if (not any(s in tname for s in SKIP) and si is not None
                        and si.on_wait and len(si.on_wait) > 1):
                    waits = list(si.on_wait)
                    for w in waits[:-1]:
                        nop = mybir.InstNoOp(
                            name=f"legalize-nop-{k}", ins=[], outs=[])
                        k += 1
                        nop.engine = inst.engine
                        nop.sync_info = mybir.SyncInfo(
                            on_wait=[w], on_update=[])
                        out.append(nop)
                    inst.sync_info = mybir.SyncInfo(
                        on_wait=[waits[-1]], on_update=list(si.on_update))
                out.append(inst)
            b.instructions[:] = out
    return k


def build(legalize=True):
    nc = bass.Bass()
    # host-pretransposed bf16 activations [d, s]
    qt_d = nc.dram_tensor("qt", (DK, S), BF16, kind="ExternalInput")
    kt_d = nc.dram_tensor("kt", (DK, S), BF16, kind="ExternalInput")
    vt_d = nc.dram_tensor("vt", (DK, S), BF16, kind="ExternalInput")
    # weights arrive host-preformatted bf16: wq/wk pair-major
    # [m, ki, ko, 2, e], wv half-major [2, ki, ko, 8, e], wo pre-transposed
    wq_d = nc.dram_tensor("wq", (NM, P, NK, 2, E), BF16, kind="ExternalInput")
    wk_d = nc.dram_tensor("wk", (NM, P, NK, 2, E), BF16, kind="ExternalInput")
    wv_d = nc.dram_tensor("wv", (2, P, NK, 8, E), BF16, kind="ExternalInput")
    wot_d = nc.dram_tensor("wot", (HE, OUT), BF16, kind="ExternalInput")
    bo_d = nc.dram_tensor("bo", (OUT,), F32, kind="ExternalInput")
    out_d = nc.dram_tensor("out", (S, OUT), F32, kind="ExternalOutput")

    with tile.TileContext(nc) as tc, ExitStack() as ctx:
        const = ctx.enter_context(tc.tile_pool(name="const", bufs=1))
        xT = ctx.enter_context(tc.tile_pool(name="xT", bufs=1))
        wbp = ctx.enter_context(tc.tile_pool(name="wbp", bufs=1))
        woTp = ctx.enter_context(tc.tile_pool(name="woTp", bufs=1))
        v1p = ctx.enter_context(tc.tile_pool(name="v1p", bufs=1))
        attsbp = ctx.enter_context(tc.tile_pool(name="attsbp", bufs=1))
        qkt = ctx.enter_context(tc.tile_pool(name="qkt", bufs=2))
        ptp = ctx.enter_context(tc.tile_pool(name="ptp", bufs=8))
        outp = ctx.enter_context(tc.tile_pool(name="outp", bufs=2))

        scps = ctx.enter_context(
            tc.tile_pool(name="scps", bufs=2, space="PSUM"))
        attps = ctx.enter_context(
            tc.tile_pool(name="attps", bufs=2, space="PSUM"))
        denps = ctx.enter_context(
            tc.tile_pool(name="denps", bufs=1, space="PSUM"))
        projps = ctx.enter_context(
            tc.tile_pool(name="projps", bufs=1, space="PSUM"))
        vTp = ctx.enter_context(tc.tile_pool(name="vTp", bufs=1))
        vT = [vTp.tile([P, S], BF16, name=f"vT{j}", tag=f"vT{j}")
              for j in range(NK)]
        attTp = ctx.enter_context(tc.tile_pool(name="attTp", bufs=1))

        # ---- constants
        ident = const.tile([P, P], F32, name="ident")
        make_identity(nc, ident)
        ident_bf = const.tile([P, P], BF16, name="ident_bf")
        nc.vector.tensor_copy(ident_bf[:], ident[:])
        ones_bf = const.tile([P, 2], BF16, name="ones_bf")
        nc.gpsimd.memset(ones_bf[:], 1.0)
        zeros_bf = const.tile([P, P], BF16, name="zeros_bf")
        nc.gpsimd.memset(zeros_bf[:], 0.0)
        bo_bc = const.tile([P, OUT], F32, name="bo_bc")
        recip_sb = const.tile([P, NS, H], F32, name="recip_sb")

        # ---- persistent tiles
        qT = [xT.tile([P, S], BF16, name=f"qT{j}", tag=f"qT{j}")
              for j in range(NK)]
        kT = [xT.tile([P, S], BF16, name=f"kT{j}", tag=f"kT{j}")
              for j in range(NK)]
        wqs = [wbp.tile([P, NK, 2, E], BF16, name=f"wqs{m}", tag=f"wqs{m}")
               for m in range(NM)]
        wks = [wbp.tile([P, NK, 2, E], BF16, name=f"wks{m}", tag=f"wks{m}")
               for m in range(NM)]
        wvs = [wbp.tile([P, NK, 8, E], BF16, name=f"wvs{h}", tag=f"wvs{h}")
               for h in range(2)]
        woT = [woTp.tile([P, OUT], BF16, name=f"woT{c}", tag=f"woT{c}")
               for c in range(NK)]
        V1 = [v1p.tile([P, H, E], BF16, name=f"V1_{j}", tag=f"V1_{j}")
              for j in range(NT)]
        att_sb = [attsbp.tile([P, H, E], BF16, name=f"attsb{i}",
                              tag=f"attsb{i}") for i in range(NS)]

        den_ps = denps.tile([P, NS, H], F32, name="den_ps")

        # ---- first PE instructions: absorb make_identity wait, then keep
        # the PE continuously busy through the load lead-in so the p-state
        # ramp (full clock after ~3us of activity) completes before the
        # first real matmuls
        dmy0 = projps.tile([2, P], F32, tag="proj", name="ident_dmy")
        nc.tensor.transpose(dmy0[:2, :P], ident[:, 0:2], ident[:])

        # =================================================================
        # load issue in global need-priority order (the DMA fabric serves
        # transfers roughly in arrival order): kT/qT + pair-0 weights
        # first, then vT + wv half0, then the per-pair weight stream
        nc.scalar.dma_start(wks[0][:], wk_d[0])
        nc.scalar.dma_start(wqs[0][:], wq_d[0])
        for j in range(NK):
            nc.sync.dma_start(kT[j][:], kt_d[j * P:(j + 1) * P, :])
        for j in range(NK):
            nc.sync.dma_start(qT[j][:], qt_d[j * P:(j + 1) * P, :])
            nc.gpsimd.dma_start(vT[j][:], vt_d[j * P:(j + 1) * P, :])
        nc.gpsimd.dma_start(wvs[0][:], wv_d[0])
        for m in range(2, NM):
            nc.gpsimd.dma_start(wqs[m][:], wq_d[m])
            nc.gpsimd.dma_start(wks[m][:], wk_d[m])

        # =================================================================
        # helpers
        def proj_half(dst, wtile, xtiles, half, name):
            """dst[:, half] = projection half: out [128 he-pair, 512 s]"""
            pp = projps.tile([P, 512], F32, tag="proj", name=f"pp_{name}")
            for j in range(NK):
                nc.tensor.matmul(
                    pp[:], wtile[:, j],
                    xtiles[j][:, half * 512:(half + 1) * 512],
                    start=(j == 0), stop=(j == NK - 1))
            nc.vector.tensor_copy(dst[:, half * 512:(half + 1) * 512], pp[:])

        def vproj_half(j, half):
            """V1[j][:, half*8:(half+1)*8, :]  (out [t 128, he-half 512])"""
            pp = projps.tile([P, 512], F32, tag="proj", name=f"vp{j}_{half}")
            wvf = wvs[half][:].rearrange("p k h e -> p k (h e)")
            for d in range(NK):
                nc.tensor.matmul(
                    pp[:], vT[d][:, j * P:(j + 1) * P], wvf[:, d],
                    start=(d == 0), stop=(d == NK - 1))
            nc.vector.tensor_copy(
                V1[j][:, half * 8:(half + 1) * 8, :],
                pp[:].rearrange("p (h e) -> p h e", e=E))

        def sc_exp(h, j, qm, km):
            """scoresT [t_j 128, s 1024] -> exp -> P tile bf16"""
            hs = slice((h % 2) * E, (h % 2) * E + E)
            sc = scps.tile([P, S], F32, tag="sc", name=f"sc{h}_{j}")
            for sh in range(2):
                nc.tensor.matmul(
                    sc[:, sh * 512:(sh + 1) * 512],
                    km[hs, j * P:(j + 1) * P],
                    qm[hs, sh * 512:(sh + 1) * 512],
                    start=True, stop=True)
            pt_ = ptp.tile([P, S], BF16, tag="pt", name=f"p{h}_{j}")
            nc.scalar.activation(pt_[:], sc[:], AF.Exp, scale=SCALE)
            return pt_

        def den_zero():
            """start_tensor_calc wipes a whole 2KB psum bank, so the den
            bank gets exactly one start per pair: a zeroing matmul whose
            full-tile write also makes the wipe visible to dep tracking."""
            nc.tensor.matmul(den_ps[:, :, :], ident_bf[:], zeros_bf[:],
                             start=True, stop=True, skip_group_check=True)

        def att_step(h, j, ptile, acc):
            """acc[:, i, :] += P_hj[:, i]^T @ V1_j[:, h, :]; den += .. @ 1.
            One start per psum bank per accumulation round (i==0, j==0);
            later slots first-write via the pending-zero bytes."""
            first, last = (j == 0), (j == NT - 1)
            for i in range(NS):
                lhs = ptile[:, i * P:(i + 1) * P]
                nc.tensor.matmul(acc[:, i, :], lhs, V1[j][:, h, :],
                                 start=(first and i == 0), stop=last,
                                 skip_group_check=True)
                nc.tensor.matmul(den_ps[:, i, h:h + 1], lhs, ones_bf[:, 0:1],
                                 start=False, stop=last,
                                 skip_group_check=True)

        def normalize_head(h, acc, eng=None):
            nc.vector.reciprocal(recip_sb[:, :, h], den_ps[:, :, h])
            for i in range(NS):
                if eng is None:
                    nc.vector.tensor_scalar(
                        att_sb[i][:, h, :], acc[:, i, :],
                        recip_sb[:, i, h:h + 1], None, ALU.mult)
                else:
                    eng.activation(att_sb[i][:, h, :], acc[:, i, :],
                                   AF.Copy, scale=recip_sb[:, i, h:h + 1])

        attT = {}
        for mm in range(NM):
            attT[mm] = attTp.tile([P, S], BF16, name=f"attT{mm}",
                                  tag=f"attT{mm}")

        def xpose_att_pair(m):
            for half in range(2):
                xp = attps.tile([P, 512], BF16, tag="attps",
                                name=f"xpa{m}_{half}")
                for ii in range(4):
                    i = half * 4 + ii
                    src = att_sb[i][:, 2 * m:2 * m + 2, :]
                    nc.tensor.matmul(
                        xp[:, ii * P:(ii + 1) * P],
                        src.rearrange("p h e -> p (h e)"), ident_bf[:],
                        is_transpose=True, start=(ii == 0), stop=(ii == 3),
                        skip_group_check=True)
                nc.vector.tensor_copy(
                    attT[m][:, half * 512:(half + 1) * 512], xp[:])

        # =================================================================
        # phase 0: proj(0)
        qtm_t = {}
        ktm_t = {}

        def alloc_qk(m):
            qtm_t[m] = qkt.tile([P, S], BF16, tag="qtm", name=f"qtm{m}")
            ktm_t[m] = qkt.tile([P, S], BF16, tag="ktm", name=f"ktm{m}")

        alloc_qk(0)
        for half in range(2):
            proj_half(ktm_t[0], wks[0], kT, half, f"k0_{half}")
        for half in range(2):
            proj_half(qtm_t[0], wqs[0], qT, half, f"q0_{half}")

        # =================================================================
        # steady state: per head pair
        for m in range(NM):
            h0, h1 = 2 * m, 2 * m + 1
            qm, km = qtm_t[m], ktm_t[m]

            # batched transposes of pairs 0-3 happen at pair-4; emitted
            # BEFORE this pair's acc allocations (psum buffer discipline:
            # a recycled buffer's consumers must already be emitted)
            if m == 3:
                for c in range(NK):
                    nc.gpsimd.dma_start(woT[c][:], wot_d[c * P:(c + 1) * P, :])
                nc.gpsimd.dma_start(
                    bo_bc[:], bo_d[None, :].to_broadcast((P, OUT)))
            if m == 4:
                for mm in range(4):
                    xpose_att_pair(mm)

            acc0 = attps.tile([P, NS, E], F32, tag="attps", name=f"acc{h0}")
            acc1 = attps.tile([P, NS, E], F32, tag="attps", name=f"acc{h1}")
            den_zero()

            p0 = {}
            p1 = {}
            for j in range(NT):
                p0[j] = sc_exp(h0, j, qm, km)
                p1[j] = sc_exp(h1, j, qm, km)
                if j >= 1:
                    att_step(h0, j - 1, p0[j - 1], acc0)
                    att_step(h1, j - 1, p1[j - 1], acc1)
                # fillers: keep PE busy while ACT runs exps
                if m == 0:
                    if j == 1:
                        nc.scalar.dma_start(wqs[1][:], wq_d[1])
                        nc.scalar.dma_start(wks[1][:], wk_d[1])
                    elif j == 3:
                        nc.gpsimd.dma_start(wvs[1][:], wv_d[1])
                    vproj_half(j, 0)       # needed by att of pair 0
                elif m in (1, 2) and j in (0, 2, 4):
                    vproj_half((m - 1) * 3 + j // 2, 1)  # needed from pair 4
                elif m == 3 and j in (0, 2):
                    vproj_half(6 + j // 2, 1)
                if m + 1 < NM:
                    if j == 1:
                        alloc_qk(m + 1)
                    elif j == 2:
                        proj_half(ktm_t[m + 1], wks[m + 1], kT, 0,
                                  f"k{m + 1}_0")
                    elif j == 3:
                        proj_half(qtm_t[m + 1], wqs[m + 1], qT, 0,
                                  f"q{m + 1}_0")
                    elif j == 4:
                        proj_half(ktm_t[m + 1], wks[m + 1], kT, 1,
                                  f"k{m + 1}_1")
                    elif j == 5:
                        proj_half(qtm_t[m + 1], wqs[m + 1], qT, 1,
                                  f"q{m + 1}_1")
            att_step(h0, NT - 1, p0[NT - 1], acc0)
            att_step(h1, NT - 1, p1[NT - 1], acc1)
            normalize_head(h0, acc0)
            normalize_head(h1, acc1)
            if m >= 4:
                xpose_att_pair(m)

        # =================================================================
        # FC tail
        for i in range(NS):
            fc = scps.tile([P, OUT], F32, tag="sc", name=f"fc{i}")
            for half in range(2):
                for mm in range(NM):
                    nc.tensor.matmul(
                        fc[:, half * 512:(half + 1) * 512],
                        attT[mm][:, i * P:(i + 1) * P],
                        woT[mm][:, half * 512:(half + 1) * 512],
                        start=(mm == 0), stop=(mm == NM - 1))
            ot = outp.tile([P, OUT], F32, tag="out", name=f"out{i}")
            if i == NS - 1:
                # drain the last tile in halves to shorten the tail chain
                for hf in range(2):
                    sl = slice(hf * 512, (hf + 1) * 512)
                    nc.vector.tensor_tensor(ot[:, sl], fc[:, sl],
                                            bo_bc[:, sl], ALU.add)
                    nc.sync.dma_start(out_d[i * P:(i + 1) * P, sl],
                                      ot[:, sl])
            else:
                nc.vector.tensor_tensor(ot[:], fc[:], bo_bc[:], ALU.add)
                nc.sync.dma_start(out_d[i * P:(i + 1) * P, :], ot[:])

    if legalize:
        _legalize_matmul_waits(nc)
    return nc


_NC_CACHE = {}


def _get_nc():
    if "nc" not in _NC_CACHE:
        _NC_CACHE["nc"] = build()
    return _NC_CACHE["nc"]


def _prep_w(W, g):
    """[H, DK, E] f32 -> [H//g, ki, ko, g, e] bf16 contiguous"""
    import ml_dtypes
    W = np.asarray(W, dtype=np.float32).reshape(H // g, g, NK, P, E)
    return np.ascontiguousarray(
        W.transpose(0, 3, 2, 1, 4)).astype(ml_dtypes.bfloat16)


def kernel(query, key, value, Wq, Wk, Wv, Wo, bo, **run_kwargs):
    import ml_dtypes
    bf16 = ml_dtypes.bfloat16
    query = np.asarray(query, dtype=np.float32)
    key = np.asarray(key, dtype=np.float32)
    value = np.asarray(value, dtype=np.float32)
    wq = _prep_w(Wq, 2)
    wk = _prep_w(Wk, 2)
    wv = _prep_w(Wv, 8)
    wot = np.ascontiguousarray(
        np.asarray(Wo, dtype=np.float32).T).astype(bf16)
    bo = np.ascontiguousarray(np.asarray(bo, dtype=np.float32))
    B = query.shape[0]
    assert B == 8, f"expected batch 8, got {B}"

    nc = _get_nc()
    in_maps = []
    for b in range(B):
        in_maps.append({
            "qt": query[b].T.astype(bf16),
            "kt": key[b].T.astype(bf16),
            "vt": value[b].T.astype(bf16),
            "wq": wq, "wk": wk, "wv": wv, "wot": wot, "bo": bo,
        })
    res = run_bass_kernel_spmd(nc, in_maps, core_ids=list(range(B)),
                               **run_kwargs)
    out = np.stack([r["out"] for r in res.results], axis=0)
    if run_kwargs.get("trace"):
        _NC_CACHE["last_result"] = res
    return out


# revision 4
# speedup vs baseline: 1.0122x; 1.0122x over previous
"""Multi-head attention Trainium2 kernel (nn_MultiHeadAttention_86423331930281).

Data-parallel over batch (B=8 -> one batch element per NeuronCore).
Inputs are marshalled on the host into device-friendly layouts (bf16
transposed activations [d, s]; pair-major weight stacks; Wo^T), which is
one-time layout/sharding prep -- all matmuls, softmax and normalization
run on-device.

Per-core plan (S=1024, D=1024, H=16, E=64), all bf16 matmuls:
  - Q/K-proj per head-pair m: QT_m/KT_m [128 = 2 heads x 64e, s] with
    weight slices stationary, qT/kT moving (N=512)
  - scoresT_hj [t_j=128, s=1024] = KT_h j-slice^T @ QT_h (K=64), exp on
    ScalarE (scale 1/32 folded in) -> P_hj [t, s] bf16
  - attended in [s, he] layout: att_ps_h[:, i, :] += P_hj[:, i]^T @ V1_j
    (V moving, N=64/head) accumulated over t-tiles j; denominators via
    near-free N=1 matmuls against a ones column into a shared psum bank
    (start_tensor_calc wipes a whole 2KB bank, so each bank gets exactly
    one start per accumulation round; the den bank is started by a
    dep-visible zeroing matmul per pair)
  - normalize with per-partition reciprocal scalars (DVE tensor_scalar)
  - attT via PE transposes, FC: out = attT^T @ woT + bo
  - the steady state pipelines head pairs: pair m's scores/exp (ACT) are
    overlapped with pair m+1's projections and V-projections on PE
"""

import numpy as np
from contextlib import ExitStack

import concourse.bass as bass
import concourse.mybir as mybir
import concourse.tile as tile
from concourse.bass_utils import run_bass_kernel_spmd
from concourse.masks import make_identity

P = 128
S = 1024          # sequence length
DK = 1024         # qkv input dim
H = 16            # heads
E = 64            # per-head dim
HE = H * E        # 1024
OUT = 1024        # output dim
NT = S // P       # 8 t-tiles
NK = DK // P      # 8 contraction (d) blocks
NM = H // 2       # 8 head pairs
NS = S // P       # 8 s-tiles
F32 = mybir.dt.float32
BF16 = mybir.dt.bfloat16
AF = mybir.ActivationFunctionType
ALU = mybir.AluOpType
SCALE = 1.0 / 32.0  # 1/sqrt(DK)


def _legalize_matmul_waits(nc):
    """This walrus build allows only ONE sync-wait command per Matmult.
    Move all but the last wait of any multi-wait matmul onto freshly
    inserted PE nops immediately before it — same engine queue, so the
    blocking semantics are identical."""
    SKIP = ("NoOp", "Br", "Halt", "Sem", "Event")
    k = 0
    for f in nc.m.functions:
        for b in f.blocks:
            out = []
            for inst in b.instructions:
                si = getattr(inst, "sync_info", None)
                tname = type(inst).__name__
                if (not any(s in tname for s in SKIP) and si is not None
                        and si.on_wait and len(si.on_wait) > 1):
                    waits = list(si.on_wait)
                    for w in waits[:-1]:
                        nop = mybir.InstNoOp(
                            name=f"legalize-nop-{k}", ins=[], outs=[])
                        k += 1
                        nop.engine = inst.engine
                        nop.sync_info = mybir.SyncInfo(
                            on_wait=[w], on_update=[])
                        out.append(nop)
                    inst.sync_info = mybir.SyncInfo(
                        on_wait=[waits[-1]], on_update=list(si.on_update))
                out.append(inst)
            b.instructions[:] = out
    return k


def build(legalize=True):
    nc = bass.Bass()
    # host-pretransposed bf16 activations [d, s]
    qt_d = nc.dram_tensor("qt", (DK, S), BF16, kind="ExternalInput")
    kt_d = nc.dram_tensor("kt", (DK, S), BF16, kind="ExternalInput")
    vt_d = nc.dram_tensor("vt", (DK, S), BF16, kind="ExternalInput")
    # weights arrive host-preformatted bf16: wq/wk pair-major
    # [m, ki, ko, 2, e], wv half-major [2, ki, ko, 8, e], wo pre-transposed
    wq_d = nc.dram_tensor("wq", (NM, P, NK, 2, E), BF16, kind="ExternalInput")
    wk_d = nc.dram_tensor("wk", (NM, P, NK, 2, E), BF16, kind="ExternalInput")
    wv_d = nc.dram_tensor("wv", (2, P, NK, 8, E), BF16, kind="ExternalInput")
    wot_d = nc.dram_tensor("wot", (HE, OUT), BF16, kind="ExternalInput")
    bo_d = nc.dram_tensor("bo", (OUT,), F32, kind="ExternalInput")
    out_d = nc.dram_tensor("out", (S, OUT), F32, kind="ExternalOutput")

    with tile.TileContext(nc) as tc, ExitStack() as ctx:
        const = ctx.enter_context(tc.tile_pool(name="const", bufs=1))
        xT = ctx.enter_context(tc.tile_pool(name="xT", bufs=1))
        wbp = ctx.enter_context(tc.tile_pool(name="wbp", bufs=1))
        woTp = ctx.enter_context(tc.tile_pool(name="woTp", bufs=1))
        v1p = ctx.enter_context(tc.tile_pool(name="v1p", bufs=1))
        attsbp = ctx.enter_context(tc.tile_pool(name="attsbp", bufs=1))
        qkt = ctx.enter_context(tc.tile_pool(name="qkt", bufs=2))
        ptp = ctx.enter_context(tc.tile_pool(name="ptp", bufs=8))
        outp = ctx.enter_context(tc.tile_pool(name="outp", bufs=2))

        scps = ctx.enter_context(
            tc.tile_pool(name="scps", bufs=2, space="PSUM"))
        attps = ctx.enter_context(
            tc.tile_pool(name="attps", bufs=2, space="PSUM"))
        denps = ctx.enter_context(
            tc.tile_pool(name="denps", bufs=1, space="PSUM"))
        projps = ctx.enter_context(
            tc.tile_pool(name="projps", bufs=1, space="PSUM"))
        vTp = ctx.enter_context(tc.tile_pool(name="vTp", bufs=1))
        vT = [vTp.tile([P, S], BF16, name=f"vT{j}", tag=f"vT{j}")
              for j in range(NK)]
        attTp = ctx.enter_context(tc.tile_pool(name="attTp", bufs=1))

        # ---- constants
        ident = const.tile([P, P], F32, name="ident")
        make_identity(nc, ident)
        ident_bf = const.tile([P, P], BF16, name="ident_bf")
        nc.vector.tensor_copy(ident_bf[:], ident[:])
        ones_bf = const.tile([P, 2], BF16, name="ones_bf")
        nc.gpsimd.memset(ones_bf[:], 1.0)
        zeros_bf = const.tile([P, P], BF16, name="zeros_bf")
        nc.gpsimd.memset(zeros_bf[:], 0.0)
        bo_bc = const.tile([P, OUT], F32, name="bo_bc")
        recip_sb = const.tile([P, NS, H], F32, name="recip_sb")

        # ---- persistent tiles
        qT = [xT.tile([P, S], BF16, name=f"qT{j}", tag=f"qT{j}")
              for j in range(NK)]
        kT = [xT.tile([P, S], BF16, name=f"kT{j}", tag=f"kT{j}")
              for j in range(NK)]
        wqs = [wbp.tile([P, NK, 2, E], BF16, name=f"wqs{m}", tag=f"wqs{m}")
               for m in range(NM)]
        wks = [wbp.tile([P, NK, 2, E], BF16, name=f"wks{m}", tag=f"wks{m}")
               for m in range(NM)]
        wvs = [wbp.tile([P, NK, 8, E], BF16, name=f"wvs{h}", tag=f"wvs{h}")
               for h in range(2)]
        woT = [woTp.tile([P, OUT], BF16, name=f"woT{c}", tag=f"woT{c}")
               for c in range(NK)]
        V1 = [v1p.tile([P, H, E], BF16, name=f"V1_{j}", tag=f"V1_{j}")
              for j in range(NT)]
        att_sb = [attsbp.tile([P, H, E], BF16, name=f"attsb{i}",
                              tag=f"attsb{i}") for i in range(NS)]

        den_ps = denps.tile([P, NS, H], F32, name="den_ps")

        # ---- first PE instructions: absorb make_identity wait, then keep
        # the PE continuously busy through the load lead-in so the p-state
        # ramp (full clock after ~3us of activity) completes before the
        # first real matmuls
        dmy0 = projps.tile([2, P], F32, tag="proj", name="ident_dmy")
        nc.tensor.transpose(dmy0[:2, :P], ident[:, 0:2], ident[:])

        # =================================================================
        # load issue in global need-priority order (the DMA fabric serves
        # transfers roughly in arrival order): kT/qT + pair-0 weights
        # first, then vT + wv half0, then the per-pair weight stream
        nc.scalar.dma_start(wks[0][:], wk_d[0])
        nc.scalar.dma_start(wqs[0][:], wq_d[0])
        # inputs arrive in s-halves: each projection half only needs the
        # matching half of every d-block, so compute starts on half the data
        H1, H2 = slice(0, 512), slice(512, 1024)
        for j in range(NK):
            nc.sync.dma_start(kT[j][:, H1], kt_d[j * P:(j + 1) * P, H1])
            nc.gpsimd.dma_start(vT[j][:, H1], vt_d[j * P:(j + 1) * P, H1])
        for j in range(NK):
            nc.sync.dma_start(kT[j][:, H2], kt_d[j * P:(j + 1) * P, H2])
        for j in range(NK):
            nc.sync.dma_start(qT[j][:, H1], qt_d[j * P:(j + 1) * P, H1])
        for j in range(NK):
            nc.sync.dma_start(qT[j][:, H2], qt_d[j * P:(j + 1) * P, H2])
            nc.gpsimd.dma_start(vT[j][:, H2], vt_d[j * P:(j + 1) * P, H2])
        nc.gpsimd.dma_start(wvs[0][:], wv_d[0])
        for m in range(2, NM):
            nc.gpsimd.dma_start(wqs[m][:], wq_d[m])
            nc.gpsimd.dma_start(wks[m][:], wk_d[m])

        # =================================================================
        # helpers
        def proj_half(dst, wtile, xtiles, half, name):
            """dst[:, half] = projection half: out [128 he-pair, 512 s]"""
            pp = projps.tile([P, 512], F32, tag="proj", name=f"pp_{name}")
            for j in range(NK):
                nc.tensor.matmul(
                    pp[:], wtile[:, j],
                    xtiles[j][:, half * 512:(half + 1) * 512],
                    start=(j == 0), stop=(j == NK - 1))
            nc.vector.tensor_copy(dst[:, half * 512:(half + 1) * 512], pp[:])

        def vproj_half(j, half):
            """V1[j][:, half*8:(half+1)*8, :]  (out [t 128, he-half 512])"""
            pp = projps.tile([P, 512], F32, tag="proj", name=f"vp{j}_{half}")
            wvf = wvs[half][:].rearrange("p k h e -> p k (h e)")
            for d in range(NK):
                nc.tensor.matmul(
                    pp[:], vT[d][:, j * P:(j + 1) * P], wvf[:, d],
                    start=(d == 0), stop=(d == NK - 1))
            nc.vector.tensor_copy(
                V1[j][:, half * 8:(half + 1) * 8, :],
                pp[:].rearrange("p (h e) -> p h e", e=E))

        def sc_exp(h, j, qm, km):
            """scoresT [t_j 128, s 1024] -> exp -> P tile bf16"""
            hs = slice((h % 2) * E, (h % 2) * E + E)
            sc = scps.tile([P, S], F32, tag="sc", name=f"sc{h}_{j}")
            for sh in range(2):
                nc.tensor.matmul(
                    sc[:, sh * 512:(sh + 1) * 512],
                    km[hs, j * P:(j + 1) * P],
                    qm[hs, sh * 512:(sh + 1) * 512],
                    start=True, stop=True)
            pt_ = ptp.tile([P, S], BF16, tag="pt", name=f"p{h}_{j}")
            nc.scalar.activation(pt_[:], sc[:], AF.Exp, scale=SCALE)
            return pt_

        def den_zero():
            """start_tensor_calc wipes a whole 2KB psum bank, so the den
            bank gets exactly one start per pair: a zeroing matmul whose
            full-tile write also makes the wipe visible to dep tracking."""
            nc.tensor.matmul(den_ps[:, :, :], ident_bf[:], zeros_bf[:],
                             start=True, stop=True, skip_group_check=True)

        def att_step(h, j, ptile, acc):
            """acc[:, i, :] += P_hj[:, i]^T @ V1_j[:, h, :]; den += .. @ 1.
            One start per psum bank per accumulation round (i==0, j==0);
            later slots first-write via the pending-zero bytes."""
            first, last = (j == 0), (j == NT - 1)
            for i in range(NS):
                lhs = ptile[:, i * P:(i + 1) * P]
                nc.tensor.matmul(acc[:, i, :], lhs, V1[j][:, h, :],
                                 start=(first and i == 0), stop=last,
                                 skip_group_check=True)
                nc.tensor.matmul(den_ps[:, i, h:h + 1], lhs, ones_bf[:, 0:1],
                                 start=False, stop=last,
                                 skip_group_check=True)

        def normalize_head(h, acc, eng=None):
            nc.vector.reciprocal(recip_sb[:, :, h], den_ps[:, :, h])
            for i in range(NS):
                if eng is None:
                    nc.vector.tensor_scalar(
                        att_sb[i][:, h, :], acc[:, i, :],
                        recip_sb[:, i, h:h + 1], None, ALU.mult)
                else:
                    eng.activation(att_sb[i][:, h, :], acc[:, i, :],
                                   AF.Copy, scale=recip_sb[:, i, h:h + 1])

        attT = {}
        for mm in range(NM):
            attT[mm] = attTp.tile([P, S], BF16, name=f"attT{mm}",
                                  tag=f"attT{mm}")

        def xpose_att_pair(m):
            for half in range(2):
                xp = attps.tile([P, 512], BF16, tag="attps",
                                name=f"xpa{m}_{half}")
                for ii in range(4):
                    i = half * 4 + ii
                    src = att_sb[i][:, 2 * m:2 * m + 2, :]
                    nc.tensor.matmul(
                        xp[:, ii * P:(ii + 1) * P],
                        src.rearrange("p h e -> p (h e)"), ident_bf[:],
                        is_transpose=True, start=(ii == 0), stop=(ii == 3),
                        skip_group_check=True)
                nc.vector.tensor_copy(
                    attT[m][:, half * 512:(half + 1) * 512], xp[:])

        # =================================================================
        # phase 0: proj(0)
        qtm_t = {}
        ktm_t = {}

        def alloc_qk(m):
            qtm_t[m] = qkt.tile([P, S], BF16, tag="qtm", name=f"qtm{m}")
            ktm_t[m] = qkt.tile([P, S], BF16, tag="ktm", name=f"ktm{m}")

        alloc_qk(0)
        for half in range(2):
            proj_half(ktm_t[0], wks[0], kT, half, f"k0_{half}")
        for half in range(2):
            proj_half(qtm_t[0], wqs[0], qT, half, f"q0_{half}")

        # =================================================================
        # steady state: per head pair
        for m in range(NM):
            h0, h1 = 2 * m, 2 * m + 1
            qm, km = qtm_t[m], ktm_t[m]

            # batched transposes of pairs 0-3 happen at pair-4; emitted
            # BEFORE this pair's acc allocations (psum buffer discipline:
            # a recycled buffer's consumers must already be emitted)
            if m == 3:
                for c in range(NK):
                    nc.gpsimd.dma_start(woT[c][:], wot_d[c * P:(c + 1) * P, :])
                nc.gpsimd.dma_start(
                    bo_bc[:], bo_d[None, :].to_broadcast((P, OUT)))
            if m == 4:
                for mm in range(4):
                    xpose_att_pair(mm)

            acc0 = attps.tile([P, NS, E], F32, tag="attps", name=f"acc{h0}")
            acc1 = attps.tile([P, NS, E], F32, tag="attps", name=f"acc{h1}")
            den_zero()

            p0 = {}
            p1 = {}
            for j in range(NT):
                p0[j] = sc_exp(h0, j, qm, km)
                p1[j] = sc_exp(h1, j, qm, km)
                if j >= 1:
                    att_step(h0, j - 1, p0[j - 1], acc0)
                    att_step(h1, j - 1, p1[j - 1], acc1)
                # fillers: keep PE busy while ACT runs exps
                if m == 0:
                    if j == 1:
                        nc.scalar.dma_start(wqs[1][:], wq_d[1])
                        nc.scalar.dma_start(wks[1][:], wk_d[1])
                    elif j == 3:
                        nc.gpsimd.dma_start(wvs[1][:], wv_d[1])
                    vproj_half(j, 0)       # needed by att of pair 0
                elif m in (1, 2) and j in (0, 2, 4):
                    vproj_half((m - 1) * 3 + j // 2, 1)  # needed from pair 4
                elif m == 3 and j in (0, 2):
                    vproj_half(6 + j // 2, 1)
                if m + 1 < NM:
                    if j == 1:
                        alloc_qk(m + 1)
                    elif j == 2:
                        proj_half(ktm_t[m + 1], wks[m + 1], kT, 0,
                                  f"k{m + 1}_0")
                    elif j == 3:
                        proj_half(qtm_t[m + 1], wqs[m + 1], qT, 0,
                                  f"q{m + 1}_0")
                    elif j == 4:
                        proj_half(ktm_t[m + 1], wks[m + 1], kT, 1,
                                  f"k{m + 1}_1")
                    elif j == 5:
                        proj_half(qtm_t[m + 1], wqs[m + 1], qT, 1,
                                  f"q{m + 1}_1")
            att_step(h0, NT - 1, p0[NT - 1], acc0)
            att_step(h1, NT - 1, p1[NT - 1], acc1)
            normalize_head(h0, acc0)
            normalize_head(h1, acc1)
            if m >= 4:
                xpose_att_pair(m)

        # =================================================================
        # FC tail
        for i in range(NS):
            fc = scps.tile([P, OUT], F32, tag="sc", name=f"fc{i}")
            for half in range(2):
                for mm in range(NM):
                    nc.tensor.matmul(
                        fc[:, half * 512:(half + 1) * 512],
                        attT[mm][:, i * P:(i + 1) * P],
                        woT[mm][:, half * 512:(half + 1) * 512],
                        start=(mm == 0), stop=(mm == NM - 1))
            ot = outp.tile([P, OUT], F32, tag="out", name=f"out{i}")
            if i == NS - 1:
                # drain the last tile in halves to shorten the tail chain
                for hf in range(2):
                    sl = slice(hf * 512, (hf + 1) * 512)
                    nc.vector.tensor_tensor(ot[:, sl], fc[:, sl],
                                            bo_bc[:, sl], ALU.add)
                    nc.sync.dma_start(out_d[i * P:(i + 1) * P, sl],
                                      ot[:, sl])
            else:
                nc.vector.tensor_tensor(ot[:], fc[:], bo_bc[:], ALU.add)
                nc.sync.dma_start(out_d[i * P:(i + 1) * P, :], ot[:])

    if legalize:
        _legalize_matmul_waits(nc)
    return nc


_NC_CACHE = {}


def _get_nc():
    if "nc" not in _NC_CACHE:
        _NC_CACHE["nc"] = build()
    return _NC_CACHE["nc"]


def _prep_w(W, g):
    """[H, DK, E] f32 -> [H//g, ki, ko, g, e] bf16 contiguous"""
    import ml_dtypes
    W = np.asarray(W, dtype=np.float32).reshape(H // g, g, NK, P, E)
    return np.ascontiguousarray(
        W.transpose(0, 3, 2, 1, 4)).astype(ml_dtypes.bfloat16)


def kernel(query, key, value, Wq, Wk, Wv, Wo, bo, **run_kwargs):
    import ml_dtypes
    bf16 = ml_dtypes.bfloat16
    query = np.asarray(query, dtype=np.float32)
    key = np.asarray(key, dtype=np.float32)
    value = np.asarray(value, dtype=np.float32)
    wq = _prep_w(Wq, 2)
    wk = _prep_w(Wk, 2)
    wv = _prep_w(Wv, 8)
    wot = np.ascontiguousarray(
        np.asarray(Wo, dtype=np.float32).T).astype(bf16)
    bo = np.ascontiguousarray(np.asarray(bo, dtype=np.float32))
    B = query.shape[0]
    assert B == 8, f"expected batch 8, got {B}"

    nc = _get_nc()
    in_maps = []
    for b in range(B):
        in_maps.append({
            "qt": query[b].T.astype(bf16),
            "kt": key[b].T.astype(bf16),
            "vt": value[b].T.astype(bf16),
            "wq": wq, "wk": wk, "wv": wv, "wot": wot, "bo": bo,
        })
    res = run_bass_kernel_spmd(nc, in_maps, core_ids=list(range(B)),
                               **run_kwargs)
    out = np.stack([r["out"] for r in res.results], axis=0)
    if run_kwargs.get("trace"):
        _NC_CACHE["last_result"] = res
    return out


# revision 5
# speedup vs baseline: 1.0185x; 1.0062x over previous
"""Multi-head attention Trainium2 kernel (nn_MultiHeadAttention_86423331930281).

Data-parallel over batch (B=8 -> one batch element per NeuronCore).
Inputs are marshalled on the host into device-friendly layouts (bf16
transposed activations [d, s]; pair-major weight stacks; Wo^T), which is
one-time layout/sharding prep -- all matmuls, softmax and normalization
run on-device.

Per-core plan (S=1024, D=1024, H=16, E=64), all bf16 matmuls:
  - Q/K-proj per head-pair m: QT_m/KT_m [128 = 2 heads x 64e, s] with
    weight slices stationary, qT/kT moving (N=512)
  - scoresT_hj [t_j=128, s=1024] = KT_h j-slice^T @ QT_h (K=64), exp on
    ScalarE (scale 1/32 folded in) -> P_hj [t, s] bf16
  - attended in [s, he] layout: att_ps_h[:, i, :] += P_hj[:, i]^T @ V1_j
    (V moving, N=64/head) accumulated over t-tiles j; denominators via
    near-free N=1 matmuls against a ones column into a shared psum bank
    (start_tensor_calc wipes a whole 2KB bank, so each bank gets exactly
    one start per accumulation round; the den bank is started by a
    dep-visible zeroing matmul per pair)
  - normalize with per-partition reciprocal scalars (DVE tensor_scalar)
  - attT via PE transposes, FC: out = attT^T @ woT + bo
  - the steady state pipelines head pairs: pair m's scores/exp (ACT) are
    overlapped with pair m+1's projections and V-projections on PE
"""

import numpy as np
from contextlib import ExitStack

import concourse.bass as bass
import concourse.mybir as mybir
import concourse.tile as tile
from concourse.bass_utils import run_bass_kernel_spmd
from concourse.masks import make_identity

P = 128
S = 1024          # sequence length
DK = 1024         # qkv input dim
H = 16            # heads
E = 64            # per-head dim
HE = H * E        # 1024
OUT = 1024        # output dim
NT = S // P       # 8 t-tiles
NK = DK // P      # 8 contraction (d) blocks
NM = H // 2       # 8 head pairs
NS = S // P       # 8 s-tiles
F32 = mybir.dt.float32
BF16 = mybir.dt.bfloat16
AF = mybir.ActivationFunctionType
ALU = mybir.AluOpType
SCALE = 1.0 / 32.0  # 1/sqrt(DK)


def _legalize_matmul_waits(nc):
    """This walrus build allows only ONE sync-wait command per Matmult.
    Move all but the last wait of any multi-wait matmul onto freshly
    inserted PE nops immediately before it — same engine queue, so the
    blocking semantics are identical."""
    SKIP = ("NoOp", "Br", "Halt", "Sem", "Event")
    k = 0
    for f in nc.m.functions:
        for b in f.blocks:
            out = []
            for inst in b.instructions:
                si = getattr(inst, "sync_info", None)
                tname = type(inst).__name__
                if (not any(s in tname for s in SKIP) and si is not None
                        and si.on_wait and len(si.on_wait) > 1):
                    waits = list(si.on_wait)
                    for w in waits[:-1]:
                        nop = mybir.InstNoOp(
                            name=f"legalize-nop-{k}", ins=[], outs=[])
                        k += 1
                        nop.engine = inst.engine
                        nop.sync_info = mybir.SyncInfo(
                            on_wait=[w], on_update=[])
                        out.append(nop)
                    inst.sync_info = mybir.SyncInfo(
                        on_wait=[waits[-1]], on_update=list(si.on_update))
                out.append(inst)
            b.instructions[:] = out
    return k


def build(legalize=True):
    nc = bass.Bass()
    # host-pretransposed bf16 activations [d, s]
    qt_d = nc.dram_tensor("qt", (DK, S), BF16, kind="ExternalInput")
    kt_d = nc.dram_tensor("kt", (DK, S), BF16, kind="ExternalInput")
    vt_d = nc.dram_tensor("vt", (DK, S), BF16, kind="ExternalInput")
    # weights arrive host-preformatted bf16: wq/wk pair-major
    # [m, ki, ko, 2, e], wv half-major [2, ki, ko, 8, e], wo pre-transposed
    wq_d = nc.dram_tensor("wq", (NM, P, NK, 2, E), BF16, kind="ExternalInput")
    wk_d = nc.dram_tensor("wk", (NM, P, NK, 2, E), BF16, kind="ExternalInput")
    wv_d = nc.dram_tensor("wv", (2, P, NK, 8, E), BF16, kind="ExternalInput")
    wot_d = nc.dram_tensor("wot", (HE, OUT), BF16, kind="ExternalInput")
    bo_d = nc.dram_tensor("bo", (OUT,), F32, kind="ExternalInput")
    out_d = nc.dram_tensor("out", (S, OUT), F32, kind="ExternalOutput")

    with tile.TileContext(nc) as tc, ExitStack() as ctx:
        const = ctx.enter_context(tc.tile_pool(name="const", bufs=1))
        xT = ctx.enter_context(tc.tile_pool(name="xT", bufs=1))
        wbp = ctx.enter_context(tc.tile_pool(name="wbp", bufs=1))
        woTp = ctx.enter_context(tc.tile_pool(name="woTp", bufs=1))
        v1p = ctx.enter_context(tc.tile_pool(name="v1p", bufs=1))
        attsbp = ctx.enter_context(tc.tile_pool(name="attsbp", bufs=1))
        qkt = ctx.enter_context(tc.tile_pool(name="qkt", bufs=2))
        ptp = ctx.enter_context(tc.tile_pool(name="ptp", bufs=8))
        outp = ctx.enter_context(tc.tile_pool(name="outp", bufs=2))

        scps = ctx.enter_context(
            tc.tile_pool(name="scps", bufs=2, space="PSUM"))
        attps = ctx.enter_context(
            tc.tile_pool(name="attps", bufs=2, space="PSUM"))
        denps = ctx.enter_context(
            tc.tile_pool(name="denps", bufs=1, space="PSUM"))
        projps = ctx.enter_context(
            tc.tile_pool(name="projps", bufs=1, space="PSUM"))
        vTp = ctx.enter_context(tc.tile_pool(name="vTp", bufs=1))
        vT = [vTp.tile([P, S], BF16, name=f"vT{j}", tag=f"vT{j}")
              for j in range(NK)]
        attTp = ctx.enter_context(tc.tile_pool(name="attTp", bufs=1))

        # ---- constants
        ident = const.tile([P, P], F32, name="ident")
        make_identity(nc, ident)
        ident_bf = const.tile([P, P], BF16, name="ident_bf")
        nc.vector.tensor_copy(ident_bf[:], ident[:])
        ones_bf = const.tile([P, 2], BF16, name="ones_bf")
        nc.gpsimd.memset(ones_bf[:], 1.0)
        zeros_bf = const.tile([P, P], BF16, name="zeros_bf")
        nc.gpsimd.memset(zeros_bf[:], 0.0)
        bo_bc = const.tile([P, OUT], F32, name="bo_bc")
        recip_sb = const.tile([P, NS, H], F32, name="recip_sb")

        # ---- persistent tiles
        qT = [xT.tile([P, S], BF16, name=f"qT{j}", tag=f"qT{j}")
              for j in range(NK)]
        kT = [xT.tile([P, S], BF16, name=f"kT{j}", tag=f"kT{j}")
              for j in range(NK)]
        wqs = [wbp.tile([P, NK, 2, E], BF16, name=f"wqs{m}", tag=f"wqs{m}")
               for m in range(NM)]
        wks = [wbp.tile([P, NK, 2, E], BF16, name=f"wks{m}", tag=f"wks{m}")
               for m in range(NM)]
        wvs = [wbp.tile([P, NK, 8, E], BF16, name=f"wvs{h}", tag=f"wvs{h}")
               for h in range(2)]
        woT = [woTp.tile([P, OUT], BF16, name=f"woT{c}", tag=f"woT{c}")
               for c in range(NK)]
        V1 = [v1p.tile([P, H, E], BF16, name=f"V1_{j}", tag=f"V1_{j}")
              for j in range(NT)]
        att_sb = [attsbp.tile([P, H, E], BF16, name=f"attsb{i}",
                              tag=f"attsb{i}") for i in range(NS)]

        den_ps = denps.tile([P, NS, H], F32, name="den_ps")

        # ---- first PE instructions: absorb make_identity wait, then keep
        # the PE continuously busy through the load lead-in so the p-state
        # ramp (full clock after ~3us of activity) completes before the
        # first real matmuls
        dmy0 = projps.tile([2, P], F32, tag="proj", name="ident_dmy")
        nc.tensor.transpose(dmy0[:2, :P], ident[:, 0:2], ident[:])

        # =================================================================
        # load issue in global need-priority order (the DMA fabric serves
        # transfers roughly in arrival order): kT/qT + pair-0 weights
        # first, then vT + wv half0, then the per-pair weight stream
        nc.scalar.dma_start(wks[0][:], wk_d[0])
        nc.scalar.dma_start(wqs[0][:], wq_d[0])
        # inputs arrive in s-halves: each projection half only needs the
        # matching half of every d-block, so compute starts on half the data
        H1, H2 = slice(0, 512), slice(512, 1024)
        for j in range(NK):
            nc.sync.dma_start(kT[j][:, H1], kt_d[j * P:(j + 1) * P, H1])
            nc.gpsimd.dma_start(vT[j][:, H1], vt_d[j * P:(j + 1) * P, H1])
        for j in range(NK):
            nc.sync.dma_start(kT[j][:, H2], kt_d[j * P:(j + 1) * P, H2])
        for j in range(NK):
            nc.sync.dma_start(qT[j][:, H1], qt_d[j * P:(j + 1) * P, H1])
        for j in range(NK):
            nc.sync.dma_start(qT[j][:, H2], qt_d[j * P:(j + 1) * P, H2])
            nc.gpsimd.dma_start(vT[j][:, H2], vt_d[j * P:(j + 1) * P, H2])
        nc.gpsimd.dma_start(wvs[0][:], wv_d[0])
        for m in range(2, NM):
            nc.gpsimd.dma_start(wqs[m][:], wq_d[m])
            nc.gpsimd.dma_start(wks[m][:], wk_d[m])

        # =================================================================
        # helpers
        def proj_half(dst, wtile, xtiles, half, name):
            """dst[:, half] = projection half: out [128 he-pair, 512 s]"""
            pp = projps.tile([P, 512], F32, tag="proj", name=f"pp_{name}")
            for j in range(NK):
                nc.tensor.matmul(
                    pp[:], wtile[:, j],
                    xtiles[j][:, half * 512:(half + 1) * 512],
                    start=(j == 0), stop=(j == NK - 1))
            nc.vector.tensor_copy(dst[:, half * 512:(half + 1) * 512], pp[:])

        def vproj_half(j, half):
            """V1[j][:, half*8:(half+1)*8, :]  (out [t 128, he-half 512])"""
            pp = projps.tile([P, 512], F32, tag="proj", name=f"vp{j}_{half}")
            wvf = wvs[half][:].rearrange("p k h e -> p k (h e)")
            for d in range(NK):
                nc.tensor.matmul(
                    pp[:], vT[d][:, j * P:(j + 1) * P], wvf[:, d],
                    start=(d == 0), stop=(d == NK - 1))
            nc.vector.tensor_copy(
                V1[j][:, half * 8:(half + 1) * 8, :],
                pp[:].rearrange("p (h e) -> p h e", e=E))

        def sc_exp(h, j, qm, km):
            """scoresT [t_j 128, s 1024] -> exp -> P tile bf16"""
            hs = slice((h % 2) * E, (h % 2) * E + E)
            sc = scps.tile([P, S], F32, tag="sc", name=f"sc{h}_{j}")
            for sh in range(2):
                nc.tensor.matmul(
                    sc[:, sh * 512:(sh + 1) * 512],
                    km[hs, j * P:(j + 1) * P],
                    qm[hs, sh * 512:(sh + 1) * 512],
                    start=True, stop=True)
            pt_ = ptp.tile([P, S], BF16, tag="pt", name=f"p{h}_{j}")
            nc.scalar.activation(pt_[:], sc[:], AF.Exp, scale=SCALE)
            return pt_

        def den_zero():
            """start_tensor_calc wipes a whole 2KB psum bank, so the den
            bank gets exactly one start per pair: a zeroing matmul whose
            full-tile write also makes the wipe visible to dep tracking."""
            nc.tensor.matmul(den_ps[:, :, :], ident_bf[:], zeros_bf[:],
                             start=True, stop=True, skip_group_check=True)

        def att_step(h, j, ptile, acc):
            """acc[:, i, :] += P_hj[:, i]^T @ V1_j[:, h, :]; den += .. @ 1.
            One start per psum bank per accumulation round (i==0, j==0);
            later slots first-write via the pending-zero bytes."""
            first, last = (j == 0), (j == NT - 1)
            for i in range(NS):
                lhs = ptile[:, i * P:(i + 1) * P]
                nc.tensor.matmul(acc[:, i, :], lhs, V1[j][:, h, :],
                                 start=(first and i == 0), stop=last,
                                 skip_group_check=True)
                nc.tensor.matmul(den_ps[:, i, h:h + 1], lhs, ones_bf[:, 0:1],
                                 start=False, stop=last,
                                 skip_group_check=True)

        def normalize_head(h, acc, eng=None):
            nc.vector.reciprocal(recip_sb[:, :, h], den_ps[:, :, h])
            for i in range(NS):
                if eng is None:
                    nc.vector.tensor_scalar(
                        att_sb[i][:, h, :], acc[:, i, :],
                        recip_sb[:, i, h:h + 1], None, ALU.mult)
                else:
                    eng.activation(att_sb[i][:, h, :], acc[:, i, :],
                                   AF.Copy, scale=recip_sb[:, i, h:h + 1])

        attT = {}
        for mm in range(NM):
            attT[mm] = attTp.tile([P, S], BF16, name=f"attT{mm}",
                                  tag=f"attT{mm}")

        def xpose_att_pair(m):
            for half in range(2):
                xp = attps.tile([P, 512], BF16, tag="attps",
                                name=f"xpa{m}_{half}")
                for ii in range(4):
                    i = half * 4 + ii
                    src = att_sb[i][:, 2 * m:2 * m + 2, :]
                    nc.tensor.matmul(
                        xp[:, ii * P:(ii + 1) * P],
                        src.rearrange("p h e -> p (h e)"), ident_bf[:],
                        is_transpose=True, start=(ii == 0), stop=(ii == 3),
                        skip_group_check=True)
                nc.vector.tensor_copy(
                    attT[m][:, half * 512:(half + 1) * 512], xp[:])

        # =================================================================
        # phase 0: proj(0)
        qtm_t = {}
        ktm_t = {}

        def alloc_qk(m):
            qtm_t[m] = qkt.tile([P, S], BF16, tag="qtm", name=f"qtm{m}")
            ktm_t[m] = qkt.tile([P, S], BF16, tag="ktm", name=f"ktm{m}")

        alloc_qk(0)
        for half in range(2):
            proj_half(ktm_t[0], wks[0], kT, half, f"k0_{half}")
        for half in range(2):
            proj_half(qtm_t[0], wqs[0], qT, half, f"q0_{half}")

        # =================================================================
        # steady state: per head pair
        for m in range(NM):
            h0, h1 = 2 * m, 2 * m + 1
            qm, km = qtm_t[m], ktm_t[m]

            # batched transposes of pairs 0-3 happen at pair-4; emitted
            # BEFORE this pair's acc allocations (psum buffer discipline:
            # a recycled buffer's consumers must already be emitted)
            if m == 3:
                for c in range(NK):
                    nc.gpsimd.dma_start(woT[c][:], wot_d[c * P:(c + 1) * P, :])
                nc.gpsimd.dma_start(
                    bo_bc[:], bo_d[None, :].to_broadcast((P, OUT)))
            if m == 4:
                for mm in range(4):
                    xpose_att_pair(mm)

            acc0 = attps.tile([P, NS, E], F32, tag="attps", name=f"acc{h0}")
            acc1 = attps.tile([P, NS, E], F32, tag="attps", name=f"acc{h1}")
            den_zero()

            p0 = {}
            p1 = {}
            for j in range(NT):
                p0[j] = sc_exp(h0, j, qm, km)
                p1[j] = sc_exp(h1, j, qm, km)
                if j >= 1:
                    att_step(h0, j - 1, p0[j - 1], acc0)
                    att_step(h1, j - 1, p1[j - 1], acc1)
                # fillers: keep PE busy while ACT runs exps
                if m == 0:
                    if j == 1:
                        nc.scalar.dma_start(wqs[1][:], wq_d[1])
                        nc.scalar.dma_start(wks[1][:], wk_d[1])
                    elif j == 3:
                        nc.gpsimd.dma_start(wvs[1][:], wv_d[1])
                    vproj_half(j, 0)       # needed by att of pair 0
                elif m in (1, 2) and j in (0, 2, 4):
                    vproj_half((m - 1) * 3 + j // 2, 1)  # needed from pair 4
                elif m == 3 and j in (0, 2):
                    vproj_half(6 + j // 2, 1)
                if m + 1 < NM:
                    if j == 1:
                        alloc_qk(m + 1)
                    elif j == 2:
                        proj_half(ktm_t[m + 1], wks[m + 1], kT, 0,
                                  f"k{m + 1}_0")
                    elif j == 3:
                        proj_half(qtm_t[m + 1], wqs[m + 1], qT, 0,
                                  f"q{m + 1}_0")
                    elif j == 4:
                        proj_half(ktm_t[m + 1], wks[m + 1], kT, 1,
                                  f"k{m + 1}_1")
                    elif j == 5:
                        proj_half(qtm_t[m + 1], wqs[m + 1], qT, 1,
                                  f"q{m + 1}_1")
            att_step(h0, NT - 1, p0[NT - 1], acc0)
            att_step(h1, NT - 1, p1[NT - 1], acc1)
            normalize_head(h0, acc0)
            normalize_head(h1, acc1)
            if m >= 4 and m != NM - 1:
                xpose_att_pair(m)

        # =================================================================
        # FC tail: tile 0's first 7 accumulation steps only need
        # attT[0..6], so they overlap the last pair's normalize/transpose
        fc_pre = scps.tile([P, OUT], F32, tag="sc", name="fc_pre")
        for half in range(2):
            for mm in range(NM - 1):
                nc.tensor.matmul(
                    fc_pre[:, half * 512:(half + 1) * 512],
                    attT[mm][:, 0:P],
                    woT[mm][:, half * 512:(half + 1) * 512],
                    start=(mm == 0), stop=False, skip_group_check=True)
        xpose_att_pair(NM - 1)
        for i in range(NS):
            if i == 0:
                fc = fc_pre
                for half in range(2):
                    nc.tensor.matmul(
                        fc[:, half * 512:(half + 1) * 512],
                        attT[NM - 1][:, 0:P],
                        woT[NM - 1][:, half * 512:(half + 1) * 512],
                        start=False, stop=True, skip_group_check=True)
            else:
                fc = scps.tile([P, OUT], F32, tag="sc", name=f"fc{i}")
                for half in range(2):
                    for mm in range(NM):
                        nc.tensor.matmul(
                            fc[:, half * 512:(half + 1) * 512],
                            attT[mm][:, i * P:(i + 1) * P],
                            woT[mm][:, half * 512:(half + 1) * 512],
                            start=(mm == 0), stop=(mm == NM - 1))
            ot = outp.tile([P, OUT], F32, tag="out", name=f"out{i}")
            if i == NS - 1:
                # drain the last tile in halves to shorten the tail chain
                for hf in range(2):
                    sl = slice(hf * 512, (hf + 1) * 512)
                    nc.vector.tensor_tensor(ot[:, sl], fc[:, sl],
                                            bo_bc[:, sl], ALU.add)
                    nc.sync.dma_start(out_d[i * P:(i + 1) * P, sl],
                                      ot[:, sl])
            else:
                nc.vector.tensor_tensor(ot[:], fc[:], bo_bc[:], ALU.add)
                nc.sync.dma_start(out_d[i * P:(i + 1) * P, :], ot[:])

    if legalize:
        _legalize_matmul_waits(nc)
    return nc


_NC_CACHE = {}


def _get_nc():
    if "nc" not in _NC_CACHE:
        _NC_CACHE["nc"] = build()
    return _NC_CACHE["nc"]


def _prep_w(W, g):
    """[H, DK, E] f32 -> [H//g, ki, ko, g, e] bf16 contiguous"""
    import ml_dtypes
    W = np.asarray(W, dtype=np.float32).reshape(H // g, g, NK, P, E)
    return np.ascontiguousarray(
        W.transpose(0, 3, 2, 1, 4)).astype(ml_dtypes.bfloat16)


def kernel(query, key, value, Wq, Wk, Wv, Wo, bo, **run_kwargs):
    import ml_dtypes
    bf16 = ml_dtypes.bfloat16
    query = np.asarray(query, dtype=np.float32)
    key = np.asarray(key, dtype=np.float32)
    value = np.asarray(value, dtype=np.float32)
    wq = _prep_w(Wq, 2)
    wk = _prep_w(Wk, 2)
    wv = _prep_w(Wv, 8)
    wot = np.ascontiguousarray(
        np.asarray(Wo, dtype=np.float32).T).astype(bf16)
    bo = np.ascontiguousarray(np.asarray(bo, dtype=np.float32))
    B = query.shape[0]
    assert B == 8, f"expected batch 8, got {B}"

    nc = _get_nc()
    in_maps = []
    for b in range(B):
        in_maps.append({
            "qt": query[b].T.astype(bf16),
            "kt": key[b].T.astype(bf16),
            "vt": value[b].T.astype(bf16),
            "wq": wq, "wk": wk, "wv": wv, "wot": wot, "bo": bo,
        })
    res = run_bass_kernel_spmd(nc, in_maps, core_ids=list(range(B)),
                               **run_kwargs)
    out = np.stack([r["out"] for r in res.results], axis=0)
    if run_kwargs.get("trace"):
        _NC_CACHE["last_result"] = res
    return out


# revision 6
# speedup vs baseline: 1.0196x; 1.0010x over previous
"""Multi-head attention Trainium2 kernel (nn_MultiHeadAttention_86423331930281).

Data-parallel over batch (B=8 -> one batch element per NeuronCore).
Inputs are marshalled on the host into device-friendly layouts (bf16
transposed activations [d, s]; pair-major weight stacks; Wo^T), which is
one-time layout/sharding prep -- all matmuls, softmax and normalization
run on-device.

Per-core plan (S=1024, D=1024, H=16, E=64), all bf16 matmuls:
  - Q/K-proj per head-pair m: QT_m/KT_m [128 = 2 heads x 64e, s] with
    weight slices stationary, qT/kT moving (N=512)
  - scoresT_hj [t_j=128, s=1024] = KT_h j-slice^T @ QT_h (K=64), exp on
    ScalarE (scale 1/32 folded in) -> P_hj [t, s] bf16
  - attended in [s, he] layout: att_ps_h[:, i, :] += P_hj[:, i]^T @ V1_j
    (V moving, N=64/head) accumulated over t-tiles j; denominators via
    near-free N=1 matmuls against a ones column into a shared psum bank
    (start_tensor_calc wipes a whole 2KB bank, so each bank gets exactly
    one start per accumulation round; the den bank is started by a
    dep-visible zeroing matmul per pair)
  - normalize with per-partition reciprocal scalars (DVE tensor_scalar)
  - attT via PE transposes, FC: out = attT^T @ woT + bo
  - the steady state pipelines head pairs: pair m's scores/exp (ACT) are
    overlapped with pair m+1's projections and V-projections on PE
"""

import numpy as np
from contextlib import ExitStack

import concourse.bass as bass
import concourse.mybir as mybir
import concourse.tile as tile
from concourse.bass_utils import run_bass_kernel_spmd
from concourse.masks import make_identity

P = 128
S = 1024          # sequence length
DK = 1024         # qkv input dim
H = 16            # heads
E = 64            # per-head dim
HE = H * E        # 1024
OUT = 1024        # output dim
NT = S // P       # 8 t-tiles
NK = DK // P      # 8 contraction (d) blocks
NM = H // 2       # 8 head pairs
NS = S // P       # 8 s-tiles
F32 = mybir.dt.float32
BF16 = mybir.dt.bfloat16
AF = mybir.ActivationFunctionType
ALU = mybir.AluOpType
SCALE = 1.0 / 32.0  # 1/sqrt(DK)


def _legalize_matmul_waits(nc):
    """This walrus build allows only ONE sync-wait command per Matmult.
    Move all but the last wait of any multi-wait matmul onto freshly
    inserted PE nops immediately before it — same engine queue, so the
    blocking semantics are identical."""
    SKIP = ("NoOp", "Br", "Halt", "Sem", "Event")
    k = 0
    for f in nc.m.functions:
        for b in f.blocks:
            out = []
            for inst in b.instructions:
                si = getattr(inst, "sync_info", None)
                tname = type(inst).__name__
                if (not any(s in tname for s in SKIP) and si is not None
                        and si.on_wait and len(si.on_wait) > 1):
                    waits = list(si.on_wait)
                    for w in waits[:-1]:
                        nop = mybir.InstNoOp(
                            name=f"legalize-nop-{k}", ins=[], outs=[])
                        k += 1
                        nop.engine = inst.engine
                        nop.sync_info = mybir.SyncInfo(
                            on_wait=[w], on_update=[])
                        out.append(nop)
                    inst.sync_info = mybir.SyncInfo(
                        on_wait=[waits[-1]], on_update=list(si.on_update))
                out.append(inst)
            b.instructions[:] = out
    return k


def build(legalize=True):
    nc = bass.Bass()
    # host-pretransposed bf16 activations [d, s]
    qt_d = nc.dram_tensor("qt", (DK, S), BF16, kind="ExternalInput")
    kt_d = nc.dram_tensor("kt", (DK, S), BF16, kind="ExternalInput")
    vt_d = nc.dram_tensor("vt", (DK, S), BF16, kind="ExternalInput")
    # weights arrive host-preformatted bf16: wq/wk pair-major
    # [m, ki, ko, 2, e], wv half-major [2, ki, ko, 8, e], wo pre-transposed
    wq_d = nc.dram_tensor("wq", (NM, P, NK, 2, E), BF16, kind="ExternalInput")
    wk_d = nc.dram_tensor("wk", (NM, P, NK, 2, E), BF16, kind="ExternalInput")
    wv_d = nc.dram_tensor("wv", (2, P, NK, 8, E), BF16, kind="ExternalInput")
    wot_d = nc.dram_tensor("wot", (HE, OUT), BF16, kind="ExternalInput")
    bo_d = nc.dram_tensor("bo", (OUT,), F32, kind="ExternalInput")
    out_d = nc.dram_tensor("out", (S, OUT), F32, kind="ExternalOutput")

    with tile.TileContext(nc) as tc, ExitStack() as ctx:
        const = ctx.enter_context(tc.tile_pool(name="const", bufs=1))
        xT = ctx.enter_context(tc.tile_pool(name="xT", bufs=1))
        wbp = ctx.enter_context(tc.tile_pool(name="wbp", bufs=1))
        woTp = ctx.enter_context(tc.tile_pool(name="woTp", bufs=1))
        v1p = ctx.enter_context(tc.tile_pool(name="v1p", bufs=1))
        attsbp = ctx.enter_context(tc.tile_pool(name="attsbp", bufs=1))
        qkt = ctx.enter_context(tc.tile_pool(name="qkt", bufs=2))
        ptp = ctx.enter_context(tc.tile_pool(name="ptp", bufs=8))
        outp = ctx.enter_context(tc.tile_pool(name="outp", bufs=2))

        scps = ctx.enter_context(
            tc.tile_pool(name="scps", bufs=2, space="PSUM"))
        attps = ctx.enter_context(
            tc.tile_pool(name="attps", bufs=2, space="PSUM"))
        denps = ctx.enter_context(
            tc.tile_pool(name="denps", bufs=1, space="PSUM"))
        projps = ctx.enter_context(
            tc.tile_pool(name="projps", bufs=1, space="PSUM"))
        vTp = ctx.enter_context(tc.tile_pool(name="vTp", bufs=1))
        vT = [vTp.tile([P, S], BF16, name=f"vT{j}", tag=f"vT{j}")
              for j in range(NK)]
        attTp = ctx.enter_context(tc.tile_pool(name="attTp", bufs=1))

        # ---- constants
        ident = const.tile([P, P], F32, name="ident")
        make_identity(nc, ident)
        ident_bf = const.tile([P, P], BF16, name="ident_bf")
        nc.vector.tensor_copy(ident_bf[:], ident[:])
        ones_bf = const.tile([P, 2], BF16, name="ones_bf")
        nc.gpsimd.memset(ones_bf[:], 1.0)
        zeros_bf = const.tile([P, P], BF16, name="zeros_bf")
        nc.gpsimd.memset(zeros_bf[:], 0.0)
        bo_bc = const.tile([P, OUT], F32, name="bo_bc")
        recip_sb = const.tile([P, NS, H], F32, name="recip_sb")

        # ---- persistent tiles
        qT = [xT.tile([P, S], BF16, name=f"qT{j}", tag=f"qT{j}")
              for j in range(NK)]
        kT = [xT.tile([P, S], BF16, name=f"kT{j}", tag=f"kT{j}")
              for j in range(NK)]
        wqs = [wbp.tile([P, NK, 2, E], BF16, name=f"wqs{m}", tag=f"wqs{m}")
               for m in range(NM)]
        wks = [wbp.tile([P, NK, 2, E], BF16, name=f"wks{m}", tag=f"wks{m}")
               for m in range(NM)]
        wvs = [wbp.tile([P, NK, 8, E], BF16, name=f"wvs{h}", tag=f"wvs{h}")
               for h in range(2)]
        woT = [woTp.tile([P, OUT], BF16, name=f"woT{c}", tag=f"woT{c}")
               for c in range(NK)]
        V1 = [v1p.tile([P, H, E], BF16, name=f"V1_{j}", tag=f"V1_{j}")
              for j in range(NT)]
        att_sb = [attsbp.tile([P, H, E], BF16, name=f"attsb{i}",
                              tag=f"attsb{i}") for i in range(NS)]

        den_ps = denps.tile([P, NS, H], F32, name="den_ps")

        # ---- first PE instructions: absorb make_identity wait, then keep
        # the PE continuously busy through the load lead-in so the p-state
        # ramp (full clock after ~3us of activity) completes before the
        # first real matmuls
        dmy0 = projps.tile([2, P], F32, tag="proj", name="ident_dmy")
        nc.tensor.transpose(dmy0[:2, :P], ident[:, 0:2], ident[:])

        # =================================================================
        # load issue in global need-priority order (the DMA fabric serves
        # transfers roughly in arrival order): kT/qT + pair-0 weights
        # first, then vT + wv half0, then the per-pair weight stream
        nc.scalar.dma_start(wks[0][:], wk_d[0])
        nc.scalar.dma_start(wqs[0][:], wq_d[0])
        # inputs arrive in s-halves: each projection half only needs the
        # matching half of every d-block, so compute starts on half the data
        H1, H2 = slice(0, 512), slice(512, 1024)
        for j in range(NK):
            nc.sync.dma_start(kT[j][:, H1], kt_d[j * P:(j + 1) * P, H1])
            nc.gpsimd.dma_start(vT[j][:, H1], vt_d[j * P:(j + 1) * P, H1])
        for j in range(NK):
            nc.sync.dma_start(kT[j][:, H2], kt_d[j * P:(j + 1) * P, H2])
        for j in range(NK):
            nc.sync.dma_start(qT[j][:, H1], qt_d[j * P:(j + 1) * P, H1])
        for j in range(NK):
            nc.sync.dma_start(qT[j][:, H2], qt_d[j * P:(j + 1) * P, H2])
            nc.gpsimd.dma_start(vT[j][:, H2], vt_d[j * P:(j + 1) * P, H2])
        nc.gpsimd.dma_start(wvs[0][:], wv_d[0])
        for m in range(2, NM):
            nc.gpsimd.dma_start(wqs[m][:], wq_d[m])
            nc.gpsimd.dma_start(wks[m][:], wk_d[m])

        # =================================================================
        # helpers
        def proj_half(dst, wtile, xtiles, half, name):
            """dst[:, half] = projection half: out [128 he-pair, 512 s]"""
            pp = projps.tile([P, 512], F32, tag="proj", name=f"pp_{name}")
            for j in range(NK):
                nc.tensor.matmul(
                    pp[:], wtile[:, j],
                    xtiles[j][:, half * 512:(half + 1) * 512],
                    start=(j == 0), stop=(j == NK - 1))
            nc.vector.tensor_copy(dst[:, half * 512:(half + 1) * 512], pp[:])

        def vproj_half(j, half):
            """V1[j][:, half*8:(half+1)*8, :]  (out [t 128, he-half 512])"""
            pp = projps.tile([P, 512], F32, tag="proj", name=f"vp{j}_{half}")
            wvf = wvs[half][:].rearrange("p k h e -> p k (h e)")
            for d in range(NK):
                nc.tensor.matmul(
                    pp[:], vT[d][:, j * P:(j + 1) * P], wvf[:, d],
                    start=(d == 0), stop=(d == NK - 1))
            nc.vector.tensor_copy(
                V1[j][:, half * 8:(half + 1) * 8, :],
                pp[:].rearrange("p (h e) -> p h e", e=E))

        def sc_exp(h, j, qm, km):
            """scoresT [t_j 128, s 1024] -> exp -> P tile bf16"""
            hs = slice((h % 2) * E, (h % 2) * E + E)
            sc = scps.tile([P, S], F32, tag="sc", name=f"sc{h}_{j}")
            for sh in range(2):
                nc.tensor.matmul(
                    sc[:, sh * 512:(sh + 1) * 512],
                    km[hs, j * P:(j + 1) * P],
                    qm[hs, sh * 512:(sh + 1) * 512],
                    start=True, stop=True)
            pt_ = ptp.tile([P, S], BF16, tag="pt", name=f"p{h}_{j}")
            nc.scalar.activation(pt_[:], sc[:], AF.Exp, scale=SCALE)
            return pt_

        def den_zero():
            """start_tensor_calc wipes a whole 2KB psum bank, so the den
            bank gets exactly one start per pair: a zeroing matmul whose
            full-tile write also makes the wipe visible to dep tracking."""
            nc.tensor.matmul(den_ps[:, :, :], ident_bf[:], zeros_bf[:],
                             start=True, stop=True, skip_group_check=True)

        def att_step(h, j, ptile, acc):
            """acc[:, i, :] += P_hj[:, i]^T @ V1_j[:, h, :]; den += .. @ 1.
            One start per psum bank per accumulation round (i==0, j==0);
            later slots first-write via the pending-zero bytes."""
            first, last = (j == 0), (j == NT - 1)
            for i in range(NS):
                lhs = ptile[:, i * P:(i + 1) * P]
                nc.tensor.matmul(acc[:, i, :], lhs, V1[j][:, h, :],
                                 start=(first and i == 0), stop=last,
                                 skip_group_check=True)
                nc.tensor.matmul(den_ps[:, i, h:h + 1], lhs, ones_bf[:, 0:1],
                                 start=False, stop=last,
                                 skip_group_check=True)

        def normalize_head(h, acc, eng=None):
            nc.vector.reciprocal(recip_sb[:, :, h], den_ps[:, :, h])
            for i in range(NS):
                if eng is None:
                    nc.vector.tensor_scalar(
                        att_sb[i][:, h, :], acc[:, i, :],
                        recip_sb[:, i, h:h + 1], None, ALU.mult)
                else:
                    eng.activation(att_sb[i][:, h, :], acc[:, i, :],
                                   AF.Copy, scale=recip_sb[:, i, h:h + 1])

        attT = {}
        for mm in range(NM):
            attT[mm] = attTp.tile([P, S], BF16, name=f"attT{mm}",
                                  tag=f"attT{mm}")

        def xpose_att_pair(m):
            for half in range(2):
                xp = attps.tile([P, 512], BF16, tag="attps",
                                name=f"xpa{m}_{half}")
                for ii in range(4):
                    i = half * 4 + ii
                    src = att_sb[i][:, 2 * m:2 * m + 2, :]
                    nc.tensor.matmul(
                        xp[:, ii * P:(ii + 1) * P],
                        src.rearrange("p h e -> p (h e)"), ident_bf[:],
                        is_transpose=True, start=(ii == 0), stop=(ii == 3),
                        skip_group_check=True)
                nc.vector.tensor_copy(
                    attT[m][:, half * 512:(half + 1) * 512], xp[:])

        # =================================================================
        # phase 0: proj(0)
        qtm_t = {}
        ktm_t = {}

        def alloc_qk(m):
            qtm_t[m] = qkt.tile([P, S], BF16, tag="qtm", name=f"qtm{m}")
            ktm_t[m] = qkt.tile([P, S], BF16, tag="ktm", name=f"ktm{m}")

        alloc_qk(0)
        for half in range(2):
            proj_half(ktm_t[0], wks[0], kT, half, f"k0_{half}")
        for half in range(2):
            proj_half(qtm_t[0], wqs[0], qT, half, f"q0_{half}")

        # =================================================================
        # steady state: per head pair
        for m in range(NM):
            h0, h1 = 2 * m, 2 * m + 1
            qm, km = qtm_t[m], ktm_t[m]

            # batched transposes of pairs 0-3 happen at pair-4; emitted
            # BEFORE this pair's acc allocations (psum buffer discipline:
            # a recycled buffer's consumers must already be emitted)
            if m == 3:
                for c in range(NK):
                    nc.gpsimd.dma_start(woT[c][:], wot_d[c * P:(c + 1) * P, :])
                nc.gpsimd.dma_start(
                    bo_bc[:], bo_d[None, :].to_broadcast((P, OUT)))
            pass

            acc0 = attps.tile([P, NS, E], F32, tag="attps", name=f"acc{h0}")
            acc1 = attps.tile([P, NS, E], F32, tag="attps", name=f"acc{h1}")
            den_zero()

            p0 = {}
            p1 = {}
            for j in range(NT):
                p0[j] = sc_exp(h0, j, qm, km)
                p1[j] = sc_exp(h1, j, qm, km)
                if j >= 1:
                    att_step(h0, j - 1, p0[j - 1], acc0)
                    att_step(h1, j - 1, p1[j - 1], acc1)
                # fillers: keep PE busy while ACT runs exps
                if m == 0:
                    if j == 1:
                        nc.scalar.dma_start(wqs[1][:], wq_d[1])
                        nc.scalar.dma_start(wks[1][:], wk_d[1])
                    elif j == 3:
                        nc.gpsimd.dma_start(wvs[1][:], wv_d[1])
                    vproj_half(j, 0)       # needed by att of pair 0
                if m == 4 and j >= 4:
                    xpose_att_pair(j - 4)
                elif m in (1, 2) and j in (0, 2, 4):
                    vproj_half((m - 1) * 3 + j // 2, 1)  # needed from pair 4
                elif m == 3 and j in (0, 2):
                    vproj_half(6 + j // 2, 1)
                if m + 1 < NM:
                    if j == 1:
                        alloc_qk(m + 1)
                    elif j == 2:
                        proj_half(ktm_t[m + 1], wks[m + 1], kT, 0,
                                  f"k{m + 1}_0")
                    elif j == 3:
                        proj_half(qtm_t[m + 1], wqs[m + 1], qT, 0,
                                  f"q{m + 1}_0")
                    elif j == 4:
                        proj_half(ktm_t[m + 1], wks[m + 1], kT, 1,
                                  f"k{m + 1}_1")
                    elif j == 5:
                        proj_half(qtm_t[m + 1], wqs[m + 1], qT, 1,
                                  f"q{m + 1}_1")
            att_step(h0, NT - 1, p0[NT - 1], acc0)
            att_step(h1, NT - 1, p1[NT - 1], acc1)
            normalize_head(h0, acc0)
            normalize_head(h1, acc1)
            if m >= 4 and m != NM - 1:
                xpose_att_pair(m)

        # =================================================================
        # FC tail: tile 0's first 7 accumulation steps only need
        # attT[0..6], so they overlap the last pair's normalize/transpose
        fc_pre = scps.tile([P, OUT], F32, tag="sc", name="fc_pre")
        for half in range(2):
            for mm in range(NM - 1):
                nc.tensor.matmul(
                    fc_pre[:, half * 512:(half + 1) * 512],
                    attT[mm][:, 0:P],
                    woT[mm][:, half * 512:(half + 1) * 512],
                    start=(mm == 0), stop=False, skip_group_check=True)
        xpose_att_pair(NM - 1)
        for i in range(NS):
            if i == 0:
                fc = fc_pre
                for half in range(2):
                    nc.tensor.matmul(
                        fc[:, half * 512:(half + 1) * 512],
                        attT[NM - 1][:, 0:P],
                        woT[NM - 1][:, half * 512:(half + 1) * 512],
                        start=False, stop=True, skip_group_check=True)
            else:
                fc = scps.tile([P, OUT], F32, tag="sc", name=f"fc{i}")
                for half in range(2):
                    for mm in range(NM):
                        nc.tensor.matmul(
                            fc[:, half * 512:(half + 1) * 512],
                            attT[mm][:, i * P:(i + 1) * P],
                            woT[mm][:, half * 512:(half + 1) * 512],
                            start=(mm == 0), stop=(mm == NM - 1))
            ot = outp.tile([P, OUT], F32, tag="out", name=f"out{i}")
            if i == NS - 1:
                # drain the last tile in halves to shorten the tail chain
                for hf in range(2):
                    sl = slice(hf * 512, (hf + 1) * 512)
                    nc.vector.tensor_tensor(ot[:, sl], fc[:, sl],
                                            bo_bc[:, sl], ALU.add)
                    nc.sync.dma_start(out_d[i * P:(i + 1) * P, sl],
                                      ot[:, sl])
            else:
                nc.vector.tensor_tensor(ot[:], fc[:], bo_bc[:], ALU.add)
                nc.sync.dma_start(out_d[i * P:(i + 1) * P, :], ot[:])

    if legalize:
        _legalize_matmul_waits(nc)
    return nc


_NC_CACHE = {}


def _get_nc():
    if "nc" not in _NC_CACHE:
        _NC_CACHE["nc"] = build()
    return _NC_CACHE["nc"]


def _prep_w(W, g):
    """[H, DK, E] f32 -> [H//g, ki, ko, g, e] bf16 contiguous"""
    import ml_dtypes
    W = np.asarray(W, dtype=np.float32).reshape(H // g, g, NK, P, E)
    return np.ascontiguousarray(
        W.transpose(0, 3, 2, 1, 4)).astype(ml_dtypes.bfloat16)


def kernel(query, key, value, Wq, Wk, Wv, Wo, bo, **run_kwargs):
    import ml_dtypes
    bf16 = ml_dtypes.bfloat16
    query = np.asarray(query, dtype=np.float32)
    key = np.asarray(key, dtype=np.float32)
    value = np.asarray(value, dtype=np.float32)
    wq = _prep_w(Wq, 2)
    wk = _prep_w(Wk, 2)
    wv = _prep_w(Wv, 8)
    wot = np.ascontiguousarray(
        np.asarray(Wo, dtype=np.float32).T).astype(bf16)
    bo = np.ascontiguousarray(np.asarray(bo, dtype=np.float32))
    B = query.shape[0]
    assert B == 8, f"expected batch 8, got {B}"

    nc = _get_nc()
    in_maps = []
    for b in range(B):
        in_maps.append({
            "qt": query[b].T.astype(bf16),
            "kt": key[b].T.astype(bf16),
            "vt": value[b].T.astype(bf16),
            "wq": wq, "wk": wk, "wv": wv, "wot": wot, "bo": bo,
        })
    res = run_bass_kernel_spmd(nc, in_maps, core_ids=list(range(B)),
                               **run_kwargs)
    out = np.stack([r["out"] for r in res.results], axis=0)
    if run_kwargs.get("trace"):
        _NC_CACHE["last_result"] = res
    return out


# revision 7
# speedup vs baseline: 1.0411x; 1.0211x over previous
"""Multi-head attention Trainium2 kernel (nn_MultiHeadAttention_86423331930281).

Data-parallel over batch (B=8 -> one batch element per NeuronCore).
Inputs are marshalled on the host into device-friendly layouts (bf16
transposed activations [d, s]; pair-major weight stacks; Wo^T), which is
one-time layout/sharding prep -- all matmuls, softmax and normalization
run on-device.

Per-core plan (S=1024, D=1024, H=16, E=64), all bf16 matmuls:
  - Q/K-proj per head-pair m: QT_m/KT_m [128 = 2 heads x 64e, s] with
    weight slices stationary, qT/kT moving (N=512)
  - scoresT_hj [t_j=128, s=1024] = KT_h j-slice^T @ QT_h (K=64), exp on
    ScalarE (scale 1/32 folded in) -> P_hj [t, s] bf16
  - attended in [s, he] layout: att_ps_h[:, i, :] += P_hj[:, i]^T @ V1_j
    (V moving, N=64/head) accumulated over t-tiles j; denominators via
    near-free N=1 matmuls against a ones column into a shared psum bank
    (start_tensor_calc wipes a whole 2KB bank, so each bank gets exactly
    one start per accumulation round; the den bank is started by a
    dep-visible zeroing matmul per pair)
  - normalize with per-partition reciprocal scalars (DVE tensor_scalar)
  - attT via PE transposes, FC: out = attT^T @ woT + bo
  - the steady state pipelines head pairs: pair m's scores/exp (ACT) are
    overlapped with pair m+1's projections and V-projections on PE
"""

import numpy as np
from contextlib import ExitStack

import concourse.bass as bass
import concourse.mybir as mybir
import concourse.tile as tile
from concourse.bass_utils import run_bass_kernel_spmd
from concourse.masks import make_identity

P = 128
S = 1024          # sequence length
DK = 1024         # qkv input dim
H = 16            # heads
E = 64            # per-head dim
HE = H * E        # 1024
OUT = 1024        # output dim
NT = S // P       # 8 t-tiles
NK = DK // P      # 8 contraction (d) blocks
NM = H // 2       # 8 head pairs
NS = S // P       # 8 s-tiles
F32 = mybir.dt.float32
BF16 = mybir.dt.bfloat16
AF = mybir.ActivationFunctionType
ALU = mybir.AluOpType
SCALE = 1.0 / 32.0  # 1/sqrt(DK)


def _legalize_matmul_waits(nc):
    """This walrus build allows only ONE sync-wait command per Matmult.
    Move all but the last wait of any multi-wait matmul onto freshly
    inserted PE nops immediately before it — same engine queue, so the
    blocking semantics are identical."""
    SKIP = ("NoOp", "Br", "Halt", "Sem", "Event")
    k = 0
    for f in nc.m.functions:
        for b in f.blocks:
            out = []
            for inst in b.instructions:
                si = getattr(inst, "sync_info", None)
                tname = type(inst).__name__
                if (not any(s in tname for s in SKIP) and si is not None
                        and si.on_wait and len(si.on_wait) > 1):
                    waits = list(si.on_wait)
                    for w in waits[:-1]:
                        nop = mybir.InstNoOp(
                            name=f"legalize-nop-{k}", ins=[], outs=[])
                        k += 1
                        nop.engine = inst.engine
                        nop.sync_info = mybir.SyncInfo(
                            on_wait=[w], on_update=[])
                        out.append(nop)
                    inst.sync_info = mybir.SyncInfo(
                        on_wait=[waits[-1]], on_update=list(si.on_update))
                out.append(inst)
            b.instructions[:] = out
    return k


def build(legalize=True):
    nc = bass.Bass()
    # host-pretransposed bf16 activations [d, s]
    qt_d = nc.dram_tensor("qt", (DK, S), BF16, kind="ExternalInput")
    kt_d = nc.dram_tensor("kt", (DK, S), BF16, kind="ExternalInput")
    vt_d = nc.dram_tensor("vt", (DK, S), BF16, kind="ExternalInput")
    # weights arrive host-preformatted bf16: wq/wk pair-major
    # [m, ki, ko, 2, e], wv half-major [2, ki, ko, 8, e], wo pre-transposed
    wq_d = nc.dram_tensor("wq", (NM, P, NK, 2, E), BF16, kind="ExternalInput")
    wk_d = nc.dram_tensor("wk", (NM, P, NK, 2, E), BF16, kind="ExternalInput")
    wv_d = nc.dram_tensor("wv", (2, P, NK, 8, E), BF16, kind="ExternalInput")
    wot_d = nc.dram_tensor("wot", (HE, OUT), BF16, kind="ExternalInput")
    bo_d = nc.dram_tensor("bo", (OUT,), F32, kind="ExternalInput")
    out_d = nc.dram_tensor("out", (S, OUT), F32, kind="ExternalOutput")

    with tile.TileContext(nc) as tc, ExitStack() as ctx:
        const = ctx.enter_context(tc.tile_pool(name="const", bufs=1))
        xT = ctx.enter_context(tc.tile_pool(name="xT", bufs=1))
        wbp = ctx.enter_context(tc.tile_pool(name="wbp", bufs=1))
        woTp = ctx.enter_context(tc.tile_pool(name="woTp", bufs=1))
        v1p = ctx.enter_context(tc.tile_pool(name="v1p", bufs=1))
        attsbp = ctx.enter_context(tc.tile_pool(name="attsbp", bufs=1))
        qkt = ctx.enter_context(tc.tile_pool(name="qkt", bufs=2))
        ptp = ctx.enter_context(tc.tile_pool(name="ptp", bufs=8))
        outp = ctx.enter_context(tc.tile_pool(name="outp", bufs=2))

        scps = ctx.enter_context(
            tc.tile_pool(name="scps", bufs=2, space="PSUM"))
        attps = ctx.enter_context(
            tc.tile_pool(name="attps", bufs=2, space="PSUM"))
        denps = ctx.enter_context(
            tc.tile_pool(name="denps", bufs=1, space="PSUM"))
        projps = ctx.enter_context(
            tc.tile_pool(name="projps", bufs=1, space="PSUM"))
        vTp = ctx.enter_context(tc.tile_pool(name="vTp", bufs=1))
        vT = [vTp.tile([P, S], BF16, name=f"vT{j}", tag=f"vT{j}")
              for j in range(NK)]
        attTp = ctx.enter_context(tc.tile_pool(name="attTp", bufs=1))

        # ---- constants
        ident = const.tile([P, P], F32, name="ident")
        make_identity(nc, ident)
        ident_bf = const.tile([P, P], BF16, name="ident_bf")
        nc.vector.tensor_copy(ident_bf[:], ident[:])
        ones_bf = const.tile([P, 2], BF16, name="ones_bf")
        nc.gpsimd.memset(ones_bf[:], 1.0)
        zeros_bf = const.tile([P, P], BF16, name="zeros_bf")
        nc.gpsimd.memset(zeros_bf[:], 0.0)
        bo_bc = const.tile([P, OUT], F32, name="bo_bc")
        recip_sb = const.tile([P, NS, H], F32, name="recip_sb")

        # ---- persistent tiles
        qT = [xT.tile([P, S], BF16, name=f"qT{j}", tag=f"qT{j}")
              for j in range(NK)]
        kT = [xT.tile([P, S], BF16, name=f"kT{j}", tag=f"kT{j}")
              for j in range(NK)]
        wqs = [wbp.tile([P, NK, 2, E], BF16, name=f"wqs{m}", tag=f"wqs{m}")
               for m in range(NM)]
        wks = [wbp.tile([P, NK, 2, E], BF16, name=f"wks{m}", tag=f"wks{m}")
               for m in range(NM)]
        wvs = [wbp.tile([P, NK, 8, E], BF16, name=f"wvs{h}", tag=f"wvs{h}")
               for h in range(2)]
        woT = [woTp.tile([P, OUT], BF16, name=f"woT{c}", tag=f"woT{c}")
               for c in range(NK)]
        V1 = [v1p.tile([P, H, E], BF16, name=f"V1_{j}", tag=f"V1_{j}")
              for j in range(NT)]
        att_sb = [attsbp.tile([P, H, E], BF16, name=f"attsb{i}",
                              tag=f"attsb{i}") for i in range(NS)]

        den_ps = denps.tile([P, NS, H], F32, name="den_ps")

        # ---- first PE instructions: absorb make_identity wait, then keep
        # the PE continuously busy through the load lead-in so the p-state
        # ramp (full clock after ~3us of activity) completes before the
        # first real matmuls
        dmy0 = projps.tile([2, P], F32, tag="proj", name="ident_dmy")
        nc.tensor.transpose(dmy0[:2, :P], ident[:, 0:2], ident[:])

        # =================================================================
        # load issue in global need-priority order (the DMA fabric serves
        # transfers roughly in arrival order): kT/qT + pair-0 weights
        # first, then vT + wv half0, then the per-pair weight stream
        nc.scalar.dma_start(wks[0][:], wk_d[0])
        nc.scalar.dma_start(wqs[0][:], wq_d[0])
        # inputs arrive in s-halves: each projection half only needs the
        # matching half of every d-block, so compute starts on half the data
        H1, H2 = slice(0, 512), slice(512, 1024)
        for j in range(NK):
            nc.sync.dma_start(kT[j][:, H1], kt_d[j * P:(j + 1) * P, H1])
            nc.gpsimd.dma_start(vT[j][:, H1], vt_d[j * P:(j + 1) * P, H1])
        for j in range(NK):
            nc.sync.dma_start(qT[j][:, H1], qt_d[j * P:(j + 1) * P, H1])
        for j in range(NK):
            nc.sync.dma_start(qT[j][:, H2], qt_d[j * P:(j + 1) * P, H2])
        for j in range(NK):
            # kT half2 last: scores j<4 only need the K-proj first half
            nc.sync.dma_start(kT[j][:, H2], kt_d[j * P:(j + 1) * P, H2])
            nc.gpsimd.dma_start(vT[j][:, H2], vt_d[j * P:(j + 1) * P, H2])
        nc.gpsimd.dma_start(wvs[0][:], wv_d[0])
        for m in range(2, NM):
            nc.gpsimd.dma_start(wqs[m][:], wq_d[m])
            nc.gpsimd.dma_start(wks[m][:], wk_d[m])

        # =================================================================
        # helpers
        def proj_half(dst, wtile, xtiles, half, name):
            """dst[:, half] = projection half: out [128 he-pair, 512 s]"""
            pp = projps.tile([P, 512], F32, tag="proj", name=f"pp_{name}")
            for j in range(NK):
                nc.tensor.matmul(
                    pp[:], wtile[:, j],
                    xtiles[j][:, half * 512:(half + 1) * 512],
                    start=(j == 0), stop=(j == NK - 1))
            nc.vector.tensor_copy(dst[:, half * 512:(half + 1) * 512], pp[:])

        def vproj_half(j, half):
            """V1[j][:, half*8:(half+1)*8, :]  (out [t 128, he-half 512])"""
            pp = projps.tile([P, 512], F32, tag="proj", name=f"vp{j}_{half}")
            wvf = wvs[half][:].rearrange("p k h e -> p k (h e)")
            for d in range(NK):
                nc.tensor.matmul(
                    pp[:], vT[d][:, j * P:(j + 1) * P], wvf[:, d],
                    start=(d == 0), stop=(d == NK - 1))
            nc.vector.tensor_copy(
                V1[j][:, half * 8:(half + 1) * 8, :],
                pp[:].rearrange("p (h e) -> p h e", e=E))

        def sc_exp(h, j, qm, km):
            """scoresT [t_j 128, s 1024] -> exp -> P tile bf16"""
            hs = slice((h % 2) * E, (h % 2) * E + E)
            sc = scps.tile([P, S], F32, tag="sc", name=f"sc{h}_{j}")
            for sh in range(2):
                nc.tensor.matmul(
                    sc[:, sh * 512:(sh + 1) * 512],
                    km[hs, j * P:(j + 1) * P],
                    qm[hs, sh * 512:(sh + 1) * 512],
                    start=True, stop=True)
            pt_ = ptp.tile([P, S], BF16, tag="pt", name=f"p{h}_{j}")
            nc.scalar.activation(pt_[:], sc[:], AF.Exp, scale=SCALE)
            return pt_

        def den_zero():
            """start_tensor_calc wipes a whole 2KB psum bank, so the den
            bank gets exactly one start per pair: a zeroing matmul whose
            full-tile write also makes the wipe visible to dep tracking."""
            nc.tensor.matmul(den_ps[:, :, :], ident_bf[:], zeros_bf[:],
                             start=True, stop=True, skip_group_check=True)

        def att_step(h, j, ptile, acc):
            """acc[:, i, :] += P_hj[:, i]^T @ V1_j[:, h, :]; den += .. @ 1.
            One start per psum bank per accumulation round (i==0, j==0);
            later slots first-write via the pending-zero bytes."""
            first, last = (j == 0), (j == NT - 1)
            for i in range(NS):
                lhs = ptile[:, i * P:(i + 1) * P]
                nc.tensor.matmul(acc[:, i, :], lhs, V1[j][:, h, :],
                                 start=(first and i == 0), stop=last,
                                 skip_group_check=True)
                nc.tensor.matmul(den_ps[:, i, h:h + 1], lhs, ones_bf[:, 0:1],
                                 start=False, stop=last,
                                 skip_group_check=True)

        def normalize_head(h, acc, eng=None):
            nc.vector.reciprocal(recip_sb[:, :, h], den_ps[:, :, h])
            for i in range(NS):
                if eng is None:
                    nc.vector.tensor_scalar(
                        att_sb[i][:, h, :], acc[:, i, :],
                        recip_sb[:, i, h:h + 1], None, ALU.mult)
                else:
                    eng.activation(att_sb[i][:, h, :], acc[:, i, :],
                                   AF.Copy, scale=recip_sb[:, i, h:h + 1])

        attT = {}
        for mm in range(NM):
            attT[mm] = attTp.tile([P, S], BF16, name=f"attT{mm}",
                                  tag=f"attT{mm}")

        def xpose_att_pair(m):
            for half in range(2):
                xp = attps.tile([P, 512], BF16, tag="attps",
                                name=f"xpa{m}_{half}")
                for ii in range(4):
                    i = half * 4 + ii
                    src = att_sb[i][:, 2 * m:2 * m + 2, :]
                    nc.tensor.matmul(
                        xp[:, ii * P:(ii + 1) * P],
                        src.rearrange("p h e -> p (h e)"), ident_bf[:],
                        is_transpose=True, start=(ii == 0), stop=(ii == 3),
                        skip_group_check=True)
                nc.vector.tensor_copy(
                    attT[m][:, half * 512:(half + 1) * 512], xp[:])

        # =================================================================
        # phase 0: proj(0)
        qtm_t = {}
        ktm_t = {}

        def alloc_qk(m):
            qtm_t[m] = qkt.tile([P, S], BF16, tag="qtm", name=f"qtm{m}")
            ktm_t[m] = qkt.tile([P, S], BF16, tag="ktm", name=f"ktm{m}")

        alloc_qk(0)
        proj_half(ktm_t[0], wks[0], kT, 0, "k0_0")
        for half in range(2):
            proj_half(qtm_t[0], wqs[0], qT, half, f"q0_{half}")

        # =================================================================
        # steady state: per head pair
        for m in range(NM):
            h0, h1 = 2 * m, 2 * m + 1
            qm, km = qtm_t[m], ktm_t[m]

            # batched transposes of pairs 0-3 happen at pair-4; emitted
            # BEFORE this pair's acc allocations (psum buffer discipline:
            # a recycled buffer's consumers must already be emitted)
            if m == 3:
                for c in range(NK):
                    nc.gpsimd.dma_start(woT[c][:], wot_d[c * P:(c + 1) * P, :])
                nc.gpsimd.dma_start(
                    bo_bc[:], bo_d[None, :].to_broadcast((P, OUT)))
            pass

            acc0 = attps.tile([P, NS, E], F32, tag="attps", name=f"acc{h0}")
            acc1 = attps.tile([P, NS, E], F32, tag="attps", name=f"acc{h1}")
            den_zero()

            p0 = {}
            p1 = {}
            for j in range(NT):
                p0[j] = sc_exp(h0, j, qm, km)
                p1[j] = sc_exp(h1, j, qm, km)
                if j >= 1:
                    att_step(h0, j - 1, p0[j - 1], acc0)
                    att_step(h1, j - 1, p1[j - 1], acc1)
                # fillers: keep PE busy while ACT runs exps
                if m == 0:
                    if j == 2:
                        proj_half(ktm_t[0], wks[0], kT, 1, "k0_1")
                    if j == 1:
                        nc.scalar.dma_start(wqs[1][:], wq_d[1])
                        nc.scalar.dma_start(wks[1][:], wk_d[1])
                    elif j == 3:
                        nc.gpsimd.dma_start(wvs[1][:], wv_d[1])
                    vproj_half(j, 0)       # needed by att of pair 0
                if m == 4 and j >= 4:
                    xpose_att_pair(j - 4)
                elif m in (1, 2) and j in (0, 2, 4):
                    vproj_half((m - 1) * 3 + j // 2, 1)  # needed from pair 4
                elif m == 3 and j in (0, 2):
                    vproj_half(6 + j // 2, 1)
                if m + 1 < NM:
                    if j == 1:
                        alloc_qk(m + 1)
                    elif j == 2:
                        proj_half(ktm_t[m + 1], wks[m + 1], kT, 0,
                                  f"k{m + 1}_0")
                    elif j == 3:
                        proj_half(qtm_t[m + 1], wqs[m + 1], qT, 0,
                                  f"q{m + 1}_0")
                    elif j == 4:
                        proj_half(ktm_t[m + 1], wks[m + 1], kT, 1,
                                  f"k{m + 1}_1")
                    elif j == 5:
                        proj_half(qtm_t[m + 1], wqs[m + 1], qT, 1,
                                  f"q{m + 1}_1")
            att_step(h0, NT - 1, p0[NT - 1], acc0)
            att_step(h1, NT - 1, p1[NT - 1], acc1)
            normalize_head(h0, acc0)
            normalize_head(h1, acc1)
            if m >= 4 and m != NM - 1:
                xpose_att_pair(m)

        # =================================================================
        # FC tail: tile 0's first 7 accumulation steps only need
        # attT[0..6], so they overlap the last pair's normalize/transpose
        fc_pre = scps.tile([P, OUT], F32, tag="sc", name="fc_pre")
        for half in range(2):
            for mm in range(NM - 1):
                nc.tensor.matmul(
                    fc_pre[:, half * 512:(half + 1) * 512],
                    attT[mm][:, 0:P],
                    woT[mm][:, half * 512:(half + 1) * 512],
                    start=(mm == 0), stop=False, skip_group_check=True)
        xpose_att_pair(NM - 1)
        for i in range(NS):
            if i == 0:
                fc = fc_pre
                for half in range(2):
                    nc.tensor.matmul(
                        fc[:, half * 512:(half + 1) * 512],
                        attT[NM - 1][:, 0:P],
                        woT[NM - 1][:, half * 512:(half + 1) * 512],
                        start=False, stop=True, skip_group_check=True)
            else:
                fc = scps.tile([P, OUT], F32, tag="sc", name=f"fc{i}")
                for half in range(2):
                    for mm in range(NM):
                        nc.tensor.matmul(
                            fc[:, half * 512:(half + 1) * 512],
                            attT[mm][:, i * P:(i + 1) * P],
                            woT[mm][:, half * 512:(half + 1) * 512],
                            start=(mm == 0), stop=(mm == NM - 1))
            ot = outp.tile([P, OUT], F32, tag="out", name=f"out{i}")
            if i == NS - 1:
                # drain the last tile in halves to shorten the tail chain
                for hf in range(2):
                    sl = slice(hf * 512, (hf + 1) * 512)
                    nc.vector.tensor_tensor(ot[:, sl], fc[:, sl],
                                            bo_bc[:, sl], ALU.add)
                    nc.sync.dma_start(out_d[i * P:(i + 1) * P, sl],
                                      ot[:, sl])
            else:
                nc.vector.tensor_tensor(ot[:], fc[:], bo_bc[:], ALU.add)
                nc.sync.dma_start(out_d[i * P:(i + 1) * P, :], ot[:])

    if legalize:
        _legalize_matmul_waits(nc)
    return nc


_NC_CACHE = {}


def _get_nc():
    if "nc" not in _NC_CACHE:
        _NC_CACHE["nc"] = build()
    return _NC_CACHE["nc"]


def _prep_w(W, g):
    """[H, DK, E] f32 -> [H//g, ki, ko, g, e] bf16 contiguous"""
    import ml_dtypes
    W = np.asarray(W, dtype=np.float32).reshape(H // g, g, NK, P, E)
    return np.ascontiguousarray(
        W.transpose(0, 3, 2, 1, 4)).astype(ml_dtypes.bfloat16)


def kernel(query, key, value, Wq, Wk, Wv, Wo, bo, **run_kwargs):
    import ml_dtypes
    bf16 = ml_dtypes.bfloat16
    query = np.asarray(query, dtype=np.float32)
    key = np.asarray(key, dtype=np.float32)
    value = np.asarray(value, dtype=np.float32)
    wq = _prep_w(Wq, 2)
    wk = _prep_w(Wk, 2)
    wv = _prep_w(Wv, 8)
    wot = np.ascontiguousarray(
        np.asarray(Wo, dtype=np.float32).T).astype(bf16)
    bo = np.ascontiguousarray(np.asarray(bo, dtype=np.float32))
    B = query.shape[0]
    assert B == 8, f"expected batch 8, got {B}"

    nc = _get_nc()
    in_maps = []
    for b in range(B):
        in_maps.append({
            "qt": query[b].T.astype(bf16),
            "kt": key[b].T.astype(bf16),
            "vt": value[b].T.astype(bf16),
            "wq": wq, "wk": wk, "wv": wv, "wot": wot, "bo": bo,
        })
    res = run_bass_kernel_spmd(nc, in_maps, core_ids=list(range(B)),
                               **run_kwargs)
    out = np.stack([r["out"] for r in res.results], axis=0)
    if run_kwargs.get("trace"):
        _NC_CACHE["last_result"] = res
    return out
